# revision 14
# baseline (speedup 1.0000x reference)
"""Delta-modulation encoder on 8 Trainium2 NeuronCores.

The reference is a sequential scan over T: recon tracks x in steps of
+-th, spikes = step direction. Parallelization: rows (b,c) are sharded
256-per-core (2 rowgroups x 128 partitions); each rowgroup's time axis is
split into U chunks of S steps, each chunk warm-started W steps early from
recon=0 (the recurrence self-synchronizes: warm and true trajectories
differ by a multiple of th and coalesce). Chunk 0's window is zero-padded
on the left, which keeps recon at exactly 0 through warmup, so every chunk
runs identical code.

Per time-step the whole core does ONE fused DVE instruction of width
2U covering all lanes of both rowgroups:

    recon' = recon + ((xq*q - recon) > th)*th - ((xq*q - recon) < -th)*th

x is shipped as int16 fixed point (q = 2^-13, clamped to +-4): the scan's
decisions only flip when x falls within q/2 of a threshold boundary
(measured: ~1.7k flips over 33.5M elements, rel err 7e-3, vs the 2e-2
gate). q is a power of two so the dequantized grid is exact in f32 and
the hardware trajectory is bit-reproducible on the host.

Input is streamed deduplicated: step i of chunk j reads x[j*S - W + i],
and the host lays x out as stream[i, j] = xpad[j*S + i] with one padded
column per rowgroup, so warmup rows are re-read from SBUF (shifted one
lane) instead of re-transferred.

Spike extraction (off the DVE critical path): rowgroup 0's recon deltas
go through Pool (tensor_tensor subtract -> fp8, sign recovered exactly on
host); rowgroup 1's recon ships via ACT as fp16 (error < th/2 for any
th >= 0.01, host differences exactly).

Correctness equals the full x-hat scan for ANY W via a host-side chain
check: the kernel ships each lane's recon entering its emit span (rw) and
at window end (rl). Chunk j is provably on the x-hat trajectory iff rw[j]
matches the corrected rl[j-1] within th/2 (real warmup gaps are multiples
of th; coalesced-but-differently-rounded walkers differ by ulps); broken
lanes are recomputed on the host from the verified checkpoint.
"""

import sys

for _p in ("/opt/trn_rl_repo",):
    if _p not in sys.path:
        sys.path.insert(0, _p)

import ml_dtypes
import numpy as np

from concourse import bacc, mybir, tile
from concourse.bass_utils import run_bass_kernel_spmd
from concourse.dve_spec import Spec, Src0, Src1, C0, C1, Zero, lower
from concourse.dve_ops import DveOp, OPS
import concourse.dve_ops as _dops
from concourse.dve_uop import DveOpSpec
from concourse.mybir import AluOpType

# ---------------------------------------------------------------- constants
B, C, T = 32, 64, 16384
N_CORES = 8
R = B * C                 # 2048 rows
RPC = R // N_CORES        # 256 rows per core (2 rowgroups x 128 partitions)
U = 256                   # time chunks per rowgroup
S = T // U                # 128 emitted steps per chunk
W = 24                    # warmup steps
L = W + S                 # processed steps per chunk
PL = 4                    # steps per piece (DMA/extraction granularity)
CW = 2 * U + 2            # stream row width: 2 rowgroups x (U + 1 pad col)
NPIN = W // PL            # pinned x pieces (re-read at steps >= S)
NPIECE = L // PL
RBUFS = 10                # x ring buffers (covers all single-use pieces)
KBUFS = 8                 # K piece buffers
SBUFS = 4                 # fp16 out staging buffers
DBUFS = 4                 # fp8 out staging buffers
SPLIT_FIRST = 2           # rows of pin piece 0 shipped in a separate first DMA
OGRP = 2                  # emit pieces per output DMA
ABL_EXTRACT = True        # ablation: emit extraction + out DMA
ABL_POOL = True           # ablation: use Pool fp8 path for rowgroup 0
QLOG = 13
QF = np.float32(2.0 ** -QLOG)
F32 = mybir.dt.float32
F16 = mybir.dt.float16
FP8 = mybir.dt.float8e4
I16 = mybir.dt.int16
assert W % PL == 0 and S % PL == 0 and W <= S and U * S == T


# ------------------------------------------------------- custom DVE op def
def _register(name, spec):
    sha = {}
    for ver in ("v3", "v4"):
        sha[ver] = DveOpSpec(
            name=name, opcode=0, uops=lower(spec, ver=ver), rd1_en=True
        ).sha(ver)
    op = DveOp(name, spec, subdim=False, uops_sha=sha)
    OPS.append(op)
    _dops.CUSTOM_DVE_SPECS[name] = spec
    _dops._SUB_OPCODE_FOR_NAME[name] = _dops._CUSTOM_DVE_ROW_BASE + len(OPS) - 1
    assert max(_dops._SUB_OPCODE_FOR_NAME.values()) < 0x20
    return op


def _dmq_ref(in0, in1, s0, s1, imm2):
    x = in0.astype(np.float32) * np.float32(s1)
    d = x - in1
    net = (d > s0).astype(np.float32) - (d < -s0).astype(np.float32)
    return in1 + net * s0


_d = Src0 * C1 - Src1
DM_STEP = _register(
    "DMQ_STEP_ANT",
    Spec(body=Src1 + ((_d > C0) - (_d < (Zero - C0))) * C0, reference=_dmq_ref),
)


# ------------------------------------------------------------ build program
def _build_program(th_val):
    nc = bacc.Bacc(None)
    xin = nc.dram_tensor("xin", [128, S * CW], I16, kind="ExternalInput")
    # rowgroup 0 spikes as fp8 recon-deltas (steps [0, S-PL]); rowgroup 1
    # recon as fp16 (steps [0, S-PL)); last piece ships raw f32 recon.
    d8t = nc.dram_tensor("d8", [128, (S - PL + 1) * U], FP8, kind="ExternalOutput")
    spk = nc.dram_tensor("spk", [128, (S - PL) * U], F16, kind="ExternalOutput")
    rwt = nc.dram_tensor("rw", [128, 2 * U], F32, kind="ExternalOutput")
    klt = nc.dram_tensor("klast", [128, PL * 2 * U], F32, kind="ExternalOutput")

    with tile.TileContext(nc) as tc:
        with (
            tc.tile_pool(name="xpin", bufs=1) as pinpool,
            tc.tile_pool(name="xring", bufs=RBUFS) as ringpool,
            tc.tile_pool(name="kp", bufs=KBUFS) as kpool,
            tc.tile_pool(name="sp", bufs=SBUFS) as spool,
            tc.tile_pool(name="dp", bufs=DBUFS) as dpool,
            tc.tile_pool(name="cp", bufs=1) as cpool,
        ):
            K0 = cpool.tile([128, 2 * U], F32)
            nc.vector.memset(K0[:], 0.0)

            pin = []
            for p in range(NPIN):
                xp = pinpool.tile([128, PL * CW], I16, tag=f"pin{p}", name=f"xp{p}")
                if p == 0 and SPLIT_FIRST:
                    sf = SPLIT_FIRST
                    nc.sync.dma_start(xp[:, 0 : sf * CW], xin[:, 0 : sf * CW])
                    nc.sync.dma_start(
                        xp[:, sf * CW : PL * CW], xin[:, sf * CW : PL * CW]
                    )
                else:
                    nc.sync.dma_start(
                        xp[:], xin[:, p * PL * CW : (p + 1) * PL * CW]
                    )
                pin.append(xp)

            def in0_ap(xt, row, off):
                # [128, 2, U] view: 2 rowgroups, U lanes, group stride U+1
                g2 = xt[:, row * CW : (row + 1) * CW].rearrange(
                    "p (g c) -> p g c", g=2
                )
                return g2[:, :, off : off + U]

            kprev = K0[:]
            kprev_tile = None  # previous K piece (for Pool boundary diff)
            for pc in range(NPIECE):
                i0 = pc * PL
                if i0 < W:
                    xt, off = pin[pc], 0
                elif i0 < S:
                    xt = ringpool.tile([128, PL * CW], I16, tag="xr", name=f"xr{pc}")
                    nc.sync.dma_start(xt[:], xin[:, i0 * CW : (i0 + PL) * CW])
                    off = 0
                else:
                    xt, off = pin[pc - S // PL], 1

                KP = kpool.tile([128, PL * 2 * U], F32, tag="k", name=f"k{pc}")
                for il in range(PL):
                    nc.vector._custom_dve(
                        DM_STEP,
                        out=KP[:, il * 2 * U : (il + 1) * 2 * U],
                        in0=in0_ap(xt, il, off),
                        in1=kprev,
                        s0=float(th_val),
                        s1=float(QF),
                    )
                    kprev = KP[:, il * 2 * U : (il + 1) * 2 * U]

                if i0 + PL == W:
                    # recon entering emit span (step W-1)
                    nc.sync.dma_start(rwt[:], KP[:, (PL - 1) * 2 * U : PL * 2 * U])
                if pc == NPIECE - 1 and ABL_EXTRACT:
                    # last piece: boundary diff on Pool (hidden under DVE),
                    # then the raw f32 recon ships whole (tail = one DMA)
                    kv = KP[:].rearrange("p (s l) -> p s l", s=PL)
                    pv = kprev_tile[:].rearrange("p (s l) -> p s l", s=PL)
                    DB = dpool.tile([128, 1, U], FP8, tag="db", name=f"db{pc}")
                    nc.gpsimd.tensor_tensor(
                        DB[:, 0:1, :],
                        kv[:, 0:1, 0:U],
                        pv[:, PL - 1 : PL, 0:U],
                        AluOpType.subtract,
                    )
                    nc.scalar.dma_start(d8t[:, (S - PL) * U :], DB[:])
                    nc.scalar.dma_start(klt[:], KP[:])
                elif i0 >= W and ABL_EXTRACT:
                    tl0 = i0 - W
                    ep = (i0 - W) // PL       # emit piece index
                    gsl = ep % OGRP           # slot within output group
                    kv = KP[:].rearrange("p (s l) -> p s l", s=PL)
                    if gsl == 0:
                        D8 = dpool.tile([128, OGRP * PL, U], FP8, tag="d", name=f"d{pc}")
                        SP = spool.tile([128, OGRP * PL, U], F16, tag="s", name=f"s{pc}")
                    r0 = gsl * PL
                    # rowgroup 0: Pool diff -> fp8
                    pv = kprev_tile[:].rearrange("p (s l) -> p s l", s=PL)
                    nc.gpsimd.tensor_tensor(
                        D8[:, r0 : r0 + 1, :],
                        kv[:, 0:1, 0:U],
                        pv[:, PL - 1 : PL, 0:U],
                        AluOpType.subtract,
                    )
                    nc.gpsimd.tensor_tensor(
                        D8[:, r0 + 1 : r0 + PL, :],
                        kv[:, 1:PL, 0:U],
                        kv[:, 0 : PL - 1, 0:U],
                        AluOpType.subtract,
                    )
                    # rowgroup 1: recon as fp16 via ACT
                    nc.scalar.activation(
                        SP[:, r0 : r0 + PL, :],
                        kv[:, :, U : 2 * U],
                        mybir.ActivationFunctionType.Copy,
                    )
                    if gsl == OGRP - 1 or pc == NPIECE - 1:
                        n = (gsl + 1) * PL
                        g0 = tl0 - gsl * PL
                        nc.scalar.dma_start(
                            d8t[:, g0 * U : (g0 + n) * U], D8[:, 0:n, :]
                        )
                        nc.scalar.dma_start(
                            spk[:, g0 * U : (g0 + n) * U], SP[:, 0:n, :]
                        )
                kprev_tile = KP
    nc.finalize()
    return nc


_NC_CACHE = {}


def _get_program(th_val):
    key = float(th_val)
    if key not in _NC_CACHE:
        _NC_CACHE[key] = _build_program(key)
    return _NC_CACHE[key]


# ------------------------------------------------------------ host helpers
def quantize(xs):
    """xs (R, T) f32 -> (int16 codes, dequantized f32 x-hat)."""
    k = np.clip(np.rint(xs * np.float32(2.0 ** QLOG)), -32767, 32767).astype(
        np.int16
    )
    return k, k.astype(np.float32) * QF


def build_xin(k_core):
    """k_core: (256, T) int16 -> xin (128, S*CW) int16.

    xin[p, i*CW + g*(U+1) + j] = kpad[g*128+p, j*S + i], kpad = k_core
    left-padded with W zeros (tail pad never consumed).
    """
    kpad = np.zeros((RPC, W + T + S), dtype=np.int16)
    kpad[:, W : W + T] = k_core
    st_r, st_e = kpad.strides
    A = np.lib.stride_tricks.as_strided(
        kpad, shape=(RPC, U + 1, S), strides=(st_r, S * st_e, st_e)
    )  # A[r, j, i] = kpad[r, j*S + i]
    out = np.empty((128, S, 2, U + 1), dtype=np.int16)
    At = A.transpose(0, 2, 1)  # (r, i, j)
    out[:, :, 0, :] = At[:128]
    out[:, :, 1, :] = At[128:]
    return np.ascontiguousarray(out.reshape(128, S * CW))


def decode_outputs(results, xq, th):
    """results: per-core dicts with 'd8' (fp8), 'spk' (fp16), 'rw','rl' (f32).
    xq: (R, T) f32 dequantized input. Returns the exact x-hat-scan spikes
    (R, T) f32."""
    th = np.float32(th)
    half = th / np.float32(2)
    out = np.empty((R, T), dtype=np.float32)
    rw = np.empty((R, U), dtype=np.float32)
    rl = np.empty((R, U), dtype=np.float32)
    for core in range(N_CORES):
        r = results[core]
        rw2 = np.asarray(r["rw"]).reshape(128, 2, U)
        # rowgroup 0: fp8 recon-deltas
        d8 = np.asarray(r["d8"]).reshape(128, S, U).astype(np.float32)
        s0 = (d8 > half).astype(np.float32) - (d8 < -half).astype(np.float32)
        # rowgroup 1: fp16 recon -> diff
        k16 = np.asarray(r["spk"]).reshape(128, S, U).astype(np.float32)
        d1 = np.empty_like(k16)
        d1[:, 0] = k16[:, 0] - rw2[:, 1]
        d1[:, 1:] = k16[:, 1:] - k16[:, :-1]
        s1 = (d1 > half).astype(np.float32) - (d1 < -half).astype(np.float32)
        blk = out[core * RPC : (core + 1) * RPC].reshape(2, 128, U, S)
        blk[0] = s0.transpose(0, 2, 1)
        blk[1] = s1.transpose(0, 2, 1)
        rw[core * RPC : (core + 1) * RPC] = rw2.transpose(1, 0, 2).reshape(RPC, U)
        rl[core * RPC : (core + 1) * RPC] = (
            np.asarray(r["rl"]).reshape(128, 2, U).transpose(1, 0, 2).reshape(RPC, U)
        )

    # ---- chain-verified fixup (see module docstring): sequential over
    # chunks (vectorized over rows), so cascaded breaks cost one pass.
    rlc = rl[:, 0].copy()  # corrected end state of the previous chunk
    outv = out.reshape(R, U, S)
    for j in range(1, U):
        bad = np.abs(rw[:, j] - rlc) > half
        if bad.any():
            rows = np.nonzero(bad)[0]
            xseg = xq[:, j * S : (j + 1) * S][rows]
            rcur = rlc[rows].copy()
            seg = np.empty((len(rows), S), dtype=np.float32)
            for i in range(S):
                dd = xseg[:, i] - rcur
                net = (dd > th).astype(np.float32) - (dd < -th).astype(np.float32)
                rcur = rcur + net * th
                seg[:, i] = net
            outv[rows, j] = seg
            rlc = rl[:, j].copy()
            rlc[rows] = rcur
        else:
            rlc = rl[:, j]
    return out


# ------------------------------------------------------------------- kernel
def kernel(x, threshold):
    x = np.ascontiguousarray(np.asarray(x, dtype=np.float32))
    th = np.float32(
        min(max(np.float32(threshold), np.float32(0.01)), np.float32(0.5))
    )
    assert x.shape == (B, C, T)

    xs = x.reshape(R, T)
    k, xq = quantize(xs)

    in_maps = []
    for core in range(N_CORES):
        xin = build_xin(k[core * RPC : (core + 1) * RPC])
        in_maps.append({"xin": xin})

    nc = _get_program(th)
    res = run_bass_kernel_spmd(nc, in_maps, list(range(N_CORES)))

    out = decode_outputs(res.results, xq, th)
    return out.reshape(B, C, T)


if __name__ == "__main__":
    rng = np.random.default_rng(0)
    xv = rng.normal(0, 1, (B, C, T)).astype(np.float32)
    o = kernel(x=xv, threshold=np.float32(0.1))
    print("kernel ran; out", o.shape, o.dtype, np.unique(o))


# revision 15
# speedup vs baseline: 1.0194x; 1.0194x over previous
"""Delta-modulation encoder on 8 Trainium2 NeuronCores.

The reference is a sequential scan over T: recon tracks x in steps of
+-th, spikes = step direction. Parallelization: rows (b,c) are sharded
256-per-core (2 rowgroups x 128 partitions); each rowgroup's time axis is
split into U chunks of S steps, each chunk warm-started W steps early from
recon=0 (the recurrence self-synchronizes: warm and true trajectories
differ by a multiple of th and coalesce). Chunk 0's window is zero-padded
on the left, which keeps recon at exactly 0 through warmup, so every chunk
runs identical code.

Per time-step the whole core does ONE fused DVE instruction of width
2U covering all lanes of both rowgroups:

    recon' = recon + ((xq*q - recon) > th)*th - ((xq*q - recon) < -th)*th

x is shipped as int16 fixed point (q = 2^-13, clamped to +-4): the scan's
decisions only flip when x falls within q/2 of a threshold boundary
(measured: ~1.7k flips over 33.5M elements, rel err 7e-3, vs the 2e-2
gate). q is a power of two so the dequantized grid is exact in f32 and
the hardware trajectory is bit-reproducible on the host.

Input is streamed deduplicated: step i of chunk j reads x[j*S - W + i],
and the host lays x out as stream[i, j] = xpad[j*S + i] with one padded
column per rowgroup, so warmup rows are re-read from SBUF (shifted one
lane) instead of re-transferred.

Spike extraction (off the DVE critical path): rowgroup 0's recon deltas
go through Pool (tensor_tensor subtract -> fp8, sign recovered exactly on
host); rowgroup 1's recon ships via ACT as fp16 (error < th/2 for any
th >= 0.01, host differences exactly).

Correctness equals the full x-hat scan for ANY W via a host-side chain
check: the kernel ships each lane's recon entering its emit span (rw) and
at window end (rl). Chunk j is provably on the x-hat trajectory iff rw[j]
matches the corrected rl[j-1] within th/2 (real warmup gaps are multiples
of th; coalesced-but-differently-rounded walkers differ by ulps); broken
lanes are recomputed on the host from the verified checkpoint.
"""

import sys

for _p in ("/opt/trn_rl_repo",):
    if _p not in sys.path:
        sys.path.insert(0, _p)

import ml_dtypes
import numpy as np

from concourse import bacc, mybir, tile
from concourse.bass_utils import run_bass_kernel_spmd
from concourse.dve_spec import Spec, Src0, Src1, C0, C1, Zero, lower
from concourse.dve_ops import DveOp, OPS
import concourse.dve_ops as _dops
from concourse.dve_uop import DveOpSpec
from concourse.mybir import AluOpType

# ---------------------------------------------------------------- constants
B, C, T = 32, 64, 16384
N_CORES = 8
R = B * C                 # 2048 rows
RPC = R // N_CORES        # 256 rows per core (2 rowgroups x 128 partitions)
U = 256                   # time chunks per rowgroup
S = T // U                # 128 emitted steps per chunk
W = 24                    # warmup steps
L = W + S                 # processed steps per chunk
PL = 4                    # steps per piece (DMA/extraction granularity)
CW = 2 * U + 2            # stream row width: 2 rowgroups x (U + 1 pad col)
NPIN = W // PL            # pinned x pieces (re-read at steps >= S)
NPIECE = L // PL
RBUFS = 10                # x ring buffers (covers all single-use pieces)
KBUFS = 8                 # K piece buffers
SBUFS = 4                 # fp16 out staging buffers
DBUFS = 4                 # fp8 out staging buffers
SPLIT_FIRST = 2           # rows of pin piece 0 shipped in a separate first DMA
OGRP = 1                  # emit pieces per output DMA
ABL_EXTRACT = True        # ablation: emit extraction + out DMA
ABL_POOL = True           # ablation: use Pool fp8 path for rowgroup 0
QLOG = 13
QF = np.float32(2.0 ** -QLOG)
F32 = mybir.dt.float32
F16 = mybir.dt.float16
FP8 = mybir.dt.float8e4
I16 = mybir.dt.int16
assert W % PL == 0 and S % PL == 0 and W <= S and U * S == T


# ------------------------------------------------------- custom DVE op def
def _register(name, spec):
    sha = {}
    for ver in ("v3", "v4"):
        sha[ver] = DveOpSpec(
            name=name, opcode=0, uops=lower(spec, ver=ver), rd1_en=True
        ).sha(ver)
    op = DveOp(name, spec, subdim=False, uops_sha=sha)
    OPS.append(op)
    _dops.CUSTOM_DVE_SPECS[name] = spec
    _dops._SUB_OPCODE_FOR_NAME[name] = _dops._CUSTOM_DVE_ROW_BASE + len(OPS) - 1
    assert max(_dops._SUB_OPCODE_FOR_NAME.values()) < 0x20
    return op


def _dmq_ref(in0, in1, s0, s1, imm2):
    x = in0.astype(np.float32) * np.float32(s1)
    d = x - in1
    net = (d > s0).astype(np.float32) - (d < -s0).astype(np.float32)
    return in1 + net * s0


_d = Src0 * C1 - Src1
DM_STEP = _register(
    "DMQ_STEP_ANT",
    Spec(body=Src1 + ((_d > C0) - (_d < (Zero - C0))) * C0, reference=_dmq_ref),
)


# ------------------------------------------------------------ build program
def _build_program(th_val):
    nc = bacc.Bacc(None)
    xin = nc.dram_tensor("xin", [128, S * CW], I16, kind="ExternalInput")
    # rowgroup 0 spikes as fp8 recon-deltas; rowgroup 1 recon as fp16
    d8t = nc.dram_tensor("d8", [128, S * U], FP8, kind="ExternalOutput")
    spk = nc.dram_tensor("spk", [128, S * U], F16, kind="ExternalOutput")
    rwt = nc.dram_tensor("rw", [128, 2 * U], F32, kind="ExternalOutput")
    rlt = nc.dram_tensor("rl", [128, 2 * U], F32, kind="ExternalOutput")

    with tile.TileContext(nc) as tc:
        with (
            tc.tile_pool(name="xpin", bufs=1) as pinpool,
            tc.tile_pool(name="xring", bufs=RBUFS) as ringpool,
            tc.tile_pool(name="kp", bufs=KBUFS) as kpool,
            tc.tile_pool(name="sp", bufs=SBUFS) as spool,
            tc.tile_pool(name="dp", bufs=DBUFS) as dpool,
            tc.tile_pool(name="cp", bufs=1) as cpool,
        ):
            K0 = cpool.tile([128, 2 * U], F32)
            nc.vector.memset(K0[:], 0.0)

            pin = []
            for p in range(NPIN):
                xp = pinpool.tile([128, PL * CW], I16, tag=f"pin{p}", name=f"xp{p}")
                if p == 0 and SPLIT_FIRST:
                    sf = SPLIT_FIRST
                    nc.sync.dma_start(xp[:, 0 : sf * CW], xin[:, 0 : sf * CW])
                    nc.sync.dma_start(
                        xp[:, sf * CW : PL * CW], xin[:, sf * CW : PL * CW]
                    )
                else:
                    nc.sync.dma_start(
                        xp[:], xin[:, p * PL * CW : (p + 1) * PL * CW]
                    )
                pin.append(xp)

            def in0_ap(xt, row, off):
                # [128, 2, U] view: 2 rowgroups, U lanes, group stride U+1
                g2 = xt[:, row * CW : (row + 1) * CW].rearrange(
                    "p (g c) -> p g c", g=2
                )
                return g2[:, :, off : off + U]

            kprev = K0[:]
            kprev_tile = None  # previous K piece (for Pool boundary diff)
            for pc in range(NPIECE):
                i0 = pc * PL
                if i0 < W:
                    xt, off = pin[pc], 0
                elif i0 < S:
                    xt = ringpool.tile([128, PL * CW], I16, tag="xr", name=f"xr{pc}")
                    nc.sync.dma_start(xt[:], xin[:, i0 * CW : (i0 + PL) * CW])
                    off = 0
                else:
                    xt, off = pin[pc - S // PL], 1

                KP = kpool.tile([128, PL * 2 * U], F32, tag="k", name=f"k{pc}")
                for il in range(PL):
                    nc.vector._custom_dve(
                        DM_STEP,
                        out=KP[:, il * 2 * U : (il + 1) * 2 * U],
                        in0=in0_ap(xt, il, off),
                        in1=kprev,
                        s0=float(th_val),
                        s1=float(QF),
                    )
                    kprev = KP[:, il * 2 * U : (il + 1) * 2 * U]

                if i0 + PL == W:
                    # recon entering emit span (step W-1)
                    nc.sync.dma_start(rwt[:], KP[:, (PL - 1) * 2 * U : PL * 2 * U])
                if i0 >= W and ABL_EXTRACT:
                    tl0 = i0 - W
                    ep = (i0 - W) // PL       # emit piece index
                    gsl = ep % OGRP           # slot within output group
                    kv = KP[:].rearrange("p (s l) -> p s l", s=PL)
                    if gsl == 0:
                        D8 = dpool.tile([128, OGRP * PL, U], FP8, tag="d", name=f"d{pc}")
                        SP = spool.tile([128, OGRP * PL, U], F16, tag="s", name=f"s{pc}")
                    r0 = gsl * PL
                    # rowgroup 0: Pool diff -> fp8
                    pv = kprev_tile[:].rearrange("p (s l) -> p s l", s=PL)
                    nc.gpsimd.tensor_tensor(
                        D8[:, r0 : r0 + 1, :],
                        kv[:, 0:1, 0:U],
                        pv[:, PL - 1 : PL, 0:U],
                        AluOpType.subtract,
                    )
                    nc.gpsimd.tensor_tensor(
                        D8[:, r0 + 1 : r0 + PL, :],
                        kv[:, 1:PL, 0:U],
                        kv[:, 0 : PL - 1, 0:U],
                        AluOpType.subtract,
                    )
                    # rowgroup 1: recon as fp16 via ACT
                    nc.scalar.activation(
                        SP[:, r0 : r0 + PL, :],
                        kv[:, :, U : 2 * U],
                        mybir.ActivationFunctionType.Copy,
                    )
                    if gsl == OGRP - 1 or pc == NPIECE - 1:
                        n = (gsl + 1) * PL
                        g0 = tl0 - gsl * PL
                        nc.scalar.dma_start(
                            d8t[:, g0 * U : (g0 + n) * U], D8[:, 0:n, :]
                        )
                        nc.scalar.dma_start(
                            spk[:, g0 * U : (g0 + n) * U], SP[:, 0:n, :]
                        )
                if pc == NPIECE - 1:
                    nc.sync.dma_start(rlt[:], KP[:, (PL - 1) * 2 * U : PL * 2 * U])
                kprev_tile = KP
    nc.finalize()
    return nc


_NC_CACHE = {}


def _get_program(th_val):
    key = float(th_val)
    if key not in _NC_CACHE:
        _NC_CACHE[key] = _build_program(key)
    return _NC_CACHE[key]


# ------------------------------------------------------------ host helpers
def quantize(xs):
    """xs (R, T) f32 -> (int16 codes, dequantized f32 x-hat)."""
    k = np.clip(np.rint(xs * np.float32(2.0 ** QLOG)), -32767, 32767).astype(
        np.int16
    )
    return k, k.astype(np.float32) * QF


def build_xin(k_core):
    """k_core: (256, T) int16 -> xin (128, S*CW) int16.

    xin[p, i*CW + g*(U+1) + j] = kpad[g*128+p, j*S + i], kpad = k_core
    left-padded with W zeros (tail pad never consumed).
    """
    kpad = np.zeros((RPC, W + T + S), dtype=np.int16)
    kpad[:, W : W + T] = k_core
    st_r, st_e = kpad.strides
    A = np.lib.stride_tricks.as_strided(
        kpad, shape=(RPC, U + 1, S), strides=(st_r, S * st_e, st_e)
    )  # A[r, j, i] = kpad[r, j*S + i]
    out = np.empty((128, S, 2, U + 1), dtype=np.int16)
    At = A.transpose(0, 2, 1)  # (r, i, j)
    out[:, :, 0, :] = At[:128]
    out[:, :, 1, :] = At[128:]
    return np.ascontiguousarray(out.reshape(128, S * CW))


def decode_outputs(results, xq, th):
    """results: per-core dicts with 'd8' (fp8), 'spk' (fp16), 'rw','rl' (f32).
    xq: (R, T) f32 dequantized input. Returns the exact x-hat-scan spikes
    (R, T) f32."""
    th = np.float32(th)
    half = th / np.float32(2)
    out = np.empty((R, T), dtype=np.float32)
    rw = np.empty((R, U), dtype=np.float32)
    rl = np.empty((R, U), dtype=np.float32)
    for core in range(N_CORES):
        r = results[core]
        rw2 = np.asarray(r["rw"]).reshape(128, 2, U)
        # rowgroup 0: fp8 recon-deltas
        d8 = np.asarray(r["d8"]).reshape(128, S, U).astype(np.float32)
        s0 = (d8 > half).astype(np.float32) - (d8 < -half).astype(np.float32)
        # rowgroup 1: fp16 recon -> diff
        k16 = np.asarray(r["spk"]).reshape(128, S, U).astype(np.float32)
        d1 = np.empty_like(k16)
        d1[:, 0] = k16[:, 0] - rw2[:, 1]
        d1[:, 1:] = k16[:, 1:] - k16[:, :-1]
        s1 = (d1 > half).astype(np.float32) - (d1 < -half).astype(np.float32)
        blk = out[core * RPC : (core + 1) * RPC].reshape(2, 128, U, S)
        blk[0] = s0.transpose(0, 2, 1)
        blk[1] = s1.transpose(0, 2, 1)
        rw[core * RPC : (core + 1) * RPC] = rw2.transpose(1, 0, 2).reshape(RPC, U)
        rl[core * RPC : (core + 1) * RPC] = (
            np.asarray(r["rl"]).reshape(128, 2, U).transpose(1, 0, 2).reshape(RPC, U)
        )

    # ---- chain-verified fixup (see module docstring): sequential over
    # chunks (vectorized over rows), so cascaded breaks cost one pass.
    rlc = rl[:, 0].copy()  # corrected end state of the previous chunk
    outv = out.reshape(R, U, S)
    for j in range(1, U):
        bad = np.abs(rw[:, j] - rlc) > half
        if bad.any():
            rows = np.nonzero(bad)[0]
            xseg = xq[:, j * S : (j + 1) * S][rows]
            rcur = rlc[rows].copy()
            seg = np.empty((len(rows), S), dtype=np.float32)
            for i in range(S):
                dd = xseg[:, i] - rcur
                net = (dd > th).astype(np.float32) - (dd < -th).astype(np.float32)
                rcur = rcur + net * th
                seg[:, i] = net
            outv[rows, j] = seg
            rlc = rl[:, j].copy()
            rlc[rows] = rcur
        else:
            rlc = rl[:, j]
    return out


# ------------------------------------------------------------------- kernel
def kernel(x, threshold):
    x = np.ascontiguousarray(np.asarray(x, dtype=np.float32))
    th = np.float32(
        min(max(np.float32(threshold), np.float32(0.01)), np.float32(0.5))
    )
    assert x.shape == (B, C, T)

    xs = x.reshape(R, T)
    k, xq = quantize(xs)

    in_maps = []
    for core in range(N_CORES):
        xin = build_xin(k[core * RPC : (core + 1) * RPC])
        in_maps.append({"xin": xin})

    nc = _get_program(th)
    res = run_bass_kernel_spmd(nc, in_maps, list(range(N_CORES)))

    out = decode_outputs(res.results, xq, th)
    return out.reshape(B, C, T)


if __name__ == "__main__":
    rng = np.random.default_rng(0)
    xv = rng.normal(0, 1, (B, C, T)).astype(np.float32)
    o = kernel(x=xv, threshold=np.float32(0.1))
    print("kernel ran; out", o.shape, o.dtype, np.unique(o))


# revision 17
# speedup vs baseline: 1.1021x; 1.0812x over previous
"""Delta-modulation encoder on 8 Trainium2 NeuronCores.

The reference is a sequential scan over T: recon tracks x in steps of
+-th, spikes = step direction. Parallelization: rows (b,c) are sharded
256-per-core (2 rowgroups x 128 partitions); each rowgroup's time axis is
split into U chunks of S steps, each chunk warm-started W steps early from
recon=0 (the recurrence self-synchronizes: warm and true trajectories
differ by a multiple of th and coalesce). Chunk 0's window is zero-padded
on the left, which keeps recon at exactly 0 through warmup, so every chunk
runs identical code.

Per time-step the whole core does ONE fused DVE instruction of width
2U covering all lanes of both rowgroups:

    recon' = recon + ((xq*q - recon) > th)*th - ((xq*q - recon) < -th)*th

x is shipped as int16 fixed point (q = 2^-13, clamped to +-4): the scan's
decisions only flip when x falls within q/2 of a threshold boundary
(measured: ~1.7k flips over 33.5M elements, rel err 7e-3, vs the 2e-2
gate). q is a power of two so the dequantized grid is exact in f32 and
the hardware trajectory is bit-reproducible on the host.

Input is streamed deduplicated: step i of chunk j reads x[j*S - W + i],
and the host lays x out as stream[i, j] = xpad[j*S + i] with one padded
column per rowgroup, so warmup rows are re-read from SBUF (shifted one
lane) instead of re-transferred.

Spike extraction (off the DVE critical path): rowgroup 0's recon deltas
go through Pool (tensor_tensor subtract -> fp8, sign recovered exactly on
host); rowgroup 1's recon ships via ACT as fp16 (error < th/2 for any
th >= 0.01, host differences exactly).

Correctness equals the full x-hat scan for ANY W via a host-side chain
check: the kernel ships each lane's recon entering its emit span (rw) and
at window end (rl). Chunk j is provably on the x-hat trajectory iff rw[j]
matches the corrected rl[j-1] within th/2 (real warmup gaps are multiples
of th; coalesced-but-differently-rounded walkers differ by ulps); broken
lanes are recomputed on the host from the verified checkpoint.
"""

import sys

for _p in ("/opt/trn_rl_repo",):
    if _p not in sys.path:
        sys.path.insert(0, _p)

import ml_dtypes
import numpy as np

from concourse import bacc, mybir, tile
from concourse.bass_utils import run_bass_kernel_spmd
from concourse.dve_spec import Spec, Src0, Src1, C0, C1, Zero, lower
from concourse.dve_ops import DveOp, OPS
import concourse.dve_ops as _dops
from concourse.dve_uop import DveOpSpec
from concourse.mybir import AluOpType

# ---------------------------------------------------------------- constants
B, C, T = 32, 64, 16384
N_CORES = 8
R = B * C                 # 2048 rows
RPC = R // N_CORES        # 256 rows per core (2 rowgroups x 128 partitions)
U = 256                   # time chunks per rowgroup
S = T // U                # emitted steps per chunk
W = 16                    # warmup steps
L = W + S                 # processed steps per chunk
PL = 4                    # steps per piece (DMA/extraction granularity)
CW = 2 * U + 2            # stream row width: 2 rowgroups x (U + 1 pad col)
NPIN = W // PL            # pinned x pieces (re-read at steps >= S)
NPIECE = L // PL
RBUFS = 12                # x ring buffers (covers all single-use pieces)
KBUFS = 8                 # K piece buffers
SBUFS = 4                 # fp16 out staging buffers
DBUFS = 4                 # fp8 out staging buffers
SPLIT_FIRST = 2           # rows of pin piece 0 shipped in a separate first DMA
OGRP = 1                  # emit pieces per output DMA
ABL_EXTRACT = True        # ablation: emit extraction + out DMA
ABL_POOL = True           # ablation: use Pool fp8 path for rowgroup 0
QLOG = 13
QF = np.float32(2.0 ** -QLOG)
F32 = mybir.dt.float32
F16 = mybir.dt.float16
FP8 = mybir.dt.float8e4
I16 = mybir.dt.int16
assert W % PL == 0 and S % PL == 0 and W <= S and U * S == T


# ------------------------------------------------------- custom DVE op def
def _register(name, spec):
    sha = {}
    for ver in ("v3", "v4"):
        sha[ver] = DveOpSpec(
            name=name, opcode=0, uops=lower(spec, ver=ver), rd1_en=True
        ).sha(ver)
    op = DveOp(name, spec, subdim=False, uops_sha=sha)
    OPS.append(op)
    _dops.CUSTOM_DVE_SPECS[name] = spec
    _dops._SUB_OPCODE_FOR_NAME[name] = _dops._CUSTOM_DVE_ROW_BASE + len(OPS) - 1
    assert max(_dops._SUB_OPCODE_FOR_NAME.values()) < 0x20
    return op


def _dmq_ref(in0, in1, s0, s1, imm2):
    x = in0.astype(np.float32) * np.float32(s1)
    d = x - in1
    net = (d > s0).astype(np.float32) - (d < -s0).astype(np.float32)
    return in1 + net * s0


_d = Src0 * C1 - Src1
DM_STEP = _register(
    "DMQ_STEP_ANT",
    Spec(body=Src1 + ((_d > C0) - (_d < (Zero - C0))) * C0, reference=_dmq_ref),
)


# ------------------------------------------------------------ build program
def _build_program(th_val):
    nc = bacc.Bacc(None)
    xin = nc.dram_tensor("xin", [128, S * CW], I16, kind="ExternalInput")
    # rowgroup 0 spikes as fp8 recon-deltas; rowgroup 1 recon as fp16
    d8t = nc.dram_tensor("d8", [128, S * U], FP8, kind="ExternalOutput")
    spk = nc.dram_tensor("spk", [128, S * U], F16, kind="ExternalOutput")
    rwt = nc.dram_tensor("rw", [128, 2 * U], F32, kind="ExternalOutput")
    rlt = nc.dram_tensor("rl", [128, 2 * U], F32, kind="ExternalOutput")

    with tile.TileContext(nc) as tc:
        with (
            tc.tile_pool(name="xpin", bufs=1) as pinpool,
            tc.tile_pool(name="xring", bufs=RBUFS) as ringpool,
            tc.tile_pool(name="kp", bufs=KBUFS) as kpool,
            tc.tile_pool(name="sp", bufs=SBUFS) as spool,
            tc.tile_pool(name="dp", bufs=DBUFS) as dpool,
            tc.tile_pool(name="cp", bufs=1) as cpool,
        ):
            K0 = cpool.tile([128, 2 * U], F32)
            nc.vector.memset(K0[:], 0.0)

            pin = []
            for p in range(NPIN):
                xp = pinpool.tile([128, PL * CW], I16, tag=f"pin{p}", name=f"xp{p}")
                if p == 0 and SPLIT_FIRST:
                    sf = SPLIT_FIRST
                    nc.sync.dma_start(xp[:, 0 : sf * CW], xin[:, 0 : sf * CW])
                    nc.sync.dma_start(
                        xp[:, sf * CW : PL * CW], xin[:, sf * CW : PL * CW]
                    )
                else:
                    nc.sync.dma_start(
                        xp[:], xin[:, p * PL * CW : (p + 1) * PL * CW]
                    )
                pin.append(xp)

            def in0_ap(xt, row, off):
                # [128, 2, U] view: 2 rowgroups, U lanes, group stride U+1
                g2 = xt[:, row * CW : (row + 1) * CW].rearrange(
                    "p (g c) -> p g c", g=2
                )
                return g2[:, :, off : off + U]

            kprev = K0[:]
            kprev_tile = None  # previous K piece (for Pool boundary diff)
            for pc in range(NPIECE):
                i0 = pc * PL
                if i0 < W:
                    xt, off = pin[pc], 0
                elif i0 < S:
                    xt = ringpool.tile([128, PL * CW], I16, tag="xr", name=f"xr{pc}")
                    nc.sync.dma_start(xt[:], xin[:, i0 * CW : (i0 + PL) * CW])
                    off = 0
                else:
                    xt, off = pin[pc - S // PL], 1

                KP = kpool.tile([128, PL * 2 * U], F32, tag="k", name=f"k{pc}")
                for il in range(PL):
                    nc.vector._custom_dve(
                        DM_STEP,
                        out=KP[:, il * 2 * U : (il + 1) * 2 * U],
                        in0=in0_ap(xt, il, off),
                        in1=kprev,
                        s0=float(th_val),
                        s1=float(QF),
                    )
                    kprev = KP[:, il * 2 * U : (il + 1) * 2 * U]

                if i0 + PL == W:
                    # recon entering emit span (step W-1)
                    nc.sync.dma_start(rwt[:], KP[:, (PL - 1) * 2 * U : PL * 2 * U])
                if i0 >= W and ABL_EXTRACT:
                    tl0 = i0 - W
                    ep = (i0 - W) // PL       # emit piece index
                    gsl = ep % OGRP           # slot within output group
                    kv = KP[:].rearrange("p (s l) -> p s l", s=PL)
                    if gsl == 0:
                        D8 = dpool.tile([128, OGRP * PL, U], FP8, tag="d", name=f"d{pc}")
                        SP = spool.tile([128, OGRP * PL, U], F16, tag="s", name=f"s{pc}")
                    r0 = gsl * PL
                    # rowgroup 0: Pool diff -> fp8
                    pv = kprev_tile[:].rearrange("p (s l) -> p s l", s=PL)
                    nc.gpsimd.tensor_tensor(
                        D8[:, r0 : r0 + 1, :],
                        kv[:, 0:1, 0:U],
                        pv[:, PL - 1 : PL, 0:U],
                        AluOpType.subtract,
                    )
                    nc.gpsimd.tensor_tensor(
                        D8[:, r0 + 1 : r0 + PL, :],
                        kv[:, 1:PL, 0:U],
                        kv[:, 0 : PL - 1, 0:U],
                        AluOpType.subtract,
                    )
                    # rowgroup 1: recon as fp16 via ACT
                    nc.scalar.activation(
                        SP[:, r0 : r0 + PL, :],
                        kv[:, :, U : 2 * U],
                        mybir.ActivationFunctionType.Copy,
                    )
                    if gsl == OGRP - 1 or pc == NPIECE - 1:
                        n = (gsl + 1) * PL
                        g0 = tl0 - gsl * PL
                        nc.scalar.dma_start(
                            d8t[:, g0 * U : (g0 + n) * U], D8[:, 0:n, :]
                        )
                        nc.scalar.dma_start(
                            spk[:, g0 * U : (g0 + n) * U], SP[:, 0:n, :]
                        )
                if pc == NPIECE - 1:
                    nc.sync.dma_start(rlt[:], KP[:, (PL - 1) * 2 * U : PL * 2 * U])
                kprev_tile = KP
    nc.finalize()
    return nc


_NC_CACHE = {}


def _get_program(th_val):
    key = float(th_val)
    if key not in _NC_CACHE:
        _NC_CACHE[key] = _build_program(key)
    return _NC_CACHE[key]


# ------------------------------------------------------------ host helpers
def quantize(xs):
    """xs (R, T) f32 -> (int16 codes, dequantized f32 x-hat)."""
    k = np.clip(np.rint(xs * np.float32(2.0 ** QLOG)), -32767, 32767).astype(
        np.int16
    )
    return k, k.astype(np.float32) * QF


def build_xin(k_core):
    """k_core: (256, T) int16 -> xin (128, S*CW) int16.

    xin[p, i*CW + g*(U+1) + j] = kpad[g*128+p, j*S + i], kpad = k_core
    left-padded with W zeros (tail pad never consumed).
    """
    kpad = np.zeros((RPC, W + T + S), dtype=np.int16)
    kpad[:, W : W + T] = k_core
    st_r, st_e = kpad.strides
    A = np.lib.stride_tricks.as_strided(
        kpad, shape=(RPC, U + 1, S), strides=(st_r, S * st_e, st_e)
    )  # A[r, j, i] = kpad[r, j*S + i]
    out = np.empty((128, S, 2, U + 1), dtype=np.int16)
    At = A.transpose(0, 2, 1)  # (r, i, j)
    out[:, :, 0, :] = At[:128]
    out[:, :, 1, :] = At[128:]
    return np.ascontiguousarray(out.reshape(128, S * CW))


def decode_outputs(results, xq, th):
    """results: per-core dicts with 'd8' (fp8), 'spk' (fp16), 'rw','rl' (f32).
    xq: (R, T) f32 dequantized input. Returns the exact x-hat-scan spikes
    (R, T) f32."""
    th = np.float32(th)
    half = th / np.float32(2)
    out = np.empty((R, T), dtype=np.float32)
    rw = np.empty((R, U), dtype=np.float32)
    rl = np.empty((R, U), dtype=np.float32)
    for core in range(N_CORES):
        r = results[core]
        rw2 = np.asarray(r["rw"]).reshape(128, 2, U)
        # rowgroup 0: fp8 recon-deltas
        d8 = np.asarray(r["d8"]).reshape(128, S, U).astype(np.float32)
        s0 = (d8 > half).astype(np.float32) - (d8 < -half).astype(np.float32)
        # rowgroup 1: fp16 recon -> diff
        k16 = np.asarray(r["spk"]).reshape(128, S, U).astype(np.float32)
        d1 = np.empty_like(k16)
        d1[:, 0] = k16[:, 0] - rw2[:, 1]
        d1[:, 1:] = k16[:, 1:] - k16[:, :-1]
        s1 = (d1 > half).astype(np.float32) - (d1 < -half).astype(np.float32)
        blk = out[core * RPC : (core + 1) * RPC].reshape(2, 128, U, S)
        blk[0] = s0.transpose(0, 2, 1)
        blk[1] = s1.transpose(0, 2, 1)
        rw[core * RPC : (core + 1) * RPC] = rw2.transpose(1, 0, 2).reshape(RPC, U)
        rl[core * RPC : (core + 1) * RPC] = (
            np.asarray(r["rl"]).reshape(128, 2, U).transpose(1, 0, 2).reshape(RPC, U)
        )

    # ---- chain-verified fixup (see module docstring): sequential over
    # chunks (vectorized over rows), so cascaded breaks cost one pass.
    rlc = rl[:, 0].copy()  # corrected end state of the previous chunk
    outv = out.reshape(R, U, S)
    for j in range(1, U):
        bad = np.abs(rw[:, j] - rlc) > half
        if bad.any():
            rows = np.nonzero(bad)[0]
            xseg = xq[:, j * S : (j + 1) * S][rows]
            rcur = rlc[rows].copy()
            seg = np.empty((len(rows), S), dtype=np.float32)
            for i in range(S):
                dd = xseg[:, i] - rcur
                net = (dd > th).astype(np.float32) - (dd < -th).astype(np.float32)
                rcur = rcur + net * th
                seg[:, i] = net
            outv[rows, j] = seg
            rlc = rl[:, j].copy()
            rlc[rows] = rcur
        else:
            rlc = rl[:, j]
    return out


# ------------------------------------------------------------------- kernel
def kernel(x, threshold):
    x = np.ascontiguousarray(np.asarray(x, dtype=np.float32))
    th = np.float32(
        min(max(np.float32(threshold), np.float32(0.01)), np.float32(0.5))
    )
    assert x.shape == (B, C, T)

    xs = x.reshape(R, T)
    k, xq = quantize(xs)

    in_maps = []
    for core in range(N_CORES):
        xin = build_xin(k[core * RPC : (core + 1) * RPC])
        in_maps.append({"xin": xin})

    nc = _get_program(th)
    res = run_bass_kernel_spmd(nc, in_maps, list(range(N_CORES)))

    out = decode_outputs(res.results, xq, th)
    return out.reshape(B, C, T)


if __name__ == "__main__":
    rng = np.random.default_rng(0)
    xv = rng.normal(0, 1, (B, C, T)).astype(np.float32)
    o = kernel(x=xv, threshold=np.float32(0.1))
    print("kernel ran; out", o.shape, o.dtype, np.unique(o))


# revision 20
# speedup vs baseline: 1.1220x; 1.0180x over previous
"""Delta-modulation encoder on 8 Trainium2 NeuronCores.

The reference is a sequential scan over T: recon tracks x in steps of
+-th, spikes = step direction. Parallelization: rows (b,c) are sharded
256-per-core (2 rowgroups x 128 partitions); each rowgroup's time axis is
split into U chunks of S steps, each chunk warm-started W steps early from
recon=0 (the recurrence self-synchronizes: warm and true trajectories
differ by a multiple of th and coalesce). Chunk 0's window is zero-padded
on the left, which keeps recon at exactly 0 through warmup, so every chunk
runs identical code.

Per time-step the whole core does ONE fused DVE instruction of width
2U covering all lanes of both rowgroups:

    recon' = recon + ((xq*q - recon) > th)*th - ((xq*q - recon) < -th)*th

x is shipped as int16 fixed point (q = 2^-13, clamped to +-4): the scan's
decisions only flip when x falls within q/2 of a threshold boundary
(measured: ~1.7k flips over 33.5M elements, rel err 7e-3, vs the 2e-2
gate). q is a power of two so the dequantized grid is exact in f32 and
the hardware trajectory is bit-reproducible on the host.

Input is streamed deduplicated: step i of chunk j reads x[j*S - W + i],
and the host lays x out as stream[i, j] = xpad[j*S + i] with one padded
column per rowgroup, so warmup rows are re-read from SBUF (shifted one
lane) instead of re-transferred.

Spike extraction (off the DVE critical path): rowgroup 0's recon deltas
go through Pool (tensor_tensor subtract -> fp8, sign recovered exactly on
host); rowgroup 1's recon ships via ACT as fp16 (error < th/2 for any
th >= 0.01, host differences exactly).

Correctness equals the full x-hat scan for ANY W via a host-side chain
check: the kernel ships each lane's recon entering its emit span (rw) and
at window end (rl). Chunk j is provably on the x-hat trajectory iff rw[j]
matches the corrected rl[j-1] within th/2 (real warmup gaps are multiples
of th; coalesced-but-differently-rounded walkers differ by ulps); broken
lanes are recomputed on the host from the verified checkpoint.
"""

import sys

for _p in ("/opt/trn_rl_repo",):
    if _p not in sys.path:
        sys.path.insert(0, _p)

import ml_dtypes
import numpy as np

from concourse import bacc, mybir, tile
from concourse.bass_utils import run_bass_kernel_spmd
from concourse.dve_spec import Spec, Src0, Src1, C0, C1, Zero, lower
from concourse.dve_ops import DveOp, OPS
import concourse.dve_ops as _dops
from concourse.dve_uop import DveOpSpec
from concourse.mybir import AluOpType

# ---------------------------------------------------------------- constants
B, C, T = 32, 64, 16384
N_CORES = 8
R = B * C                 # 2048 rows
RPC = R // N_CORES        # 256 rows per core (2 rowgroups x 128 partitions)
U = 256                   # time chunks per rowgroup
S = T // U                # emitted steps per chunk
W = 16                    # warmup steps
L = W + S                 # processed steps per chunk
PL = 4                    # steps per piece (DMA/extraction granularity)
CW = 2 * U + 2            # stream row width: 2 rowgroups x (U + 1 pad col)
NPIN = W // PL            # pinned x pieces (re-read at steps >= S)
NPIECE = L // PL
RBUFS = 8                 # x ring buffers
KBUFS = 18                # K piece buffers
SBUFS = 4                 # fp16 out staging buffers
DBUFS = 4                 # fp8 out staging buffers
SPLIT_FIRST = 2           # rows of pin piece 0 shipped in a separate first DMA
OGRP = 1                  # emit pieces per output DMA
ABL_EXTRACT = True        # ablation: emit extraction + out DMA
ABL_POOL = True           # ablation: use Pool fp8 path for rowgroup 0
QLOG = 13
QF = np.float32(2.0 ** -QLOG)
F32 = mybir.dt.float32
F16 = mybir.dt.float16
FP8 = mybir.dt.float8e4
I16 = mybir.dt.int16
assert W % PL == 0 and S % PL == 0 and W <= S and U * S == T


# ------------------------------------------------------- custom DVE op def
def _register(name, spec):
    sha = {}
    for ver in ("v3", "v4"):
        sha[ver] = DveOpSpec(
            name=name, opcode=0, uops=lower(spec, ver=ver), rd1_en=True
        ).sha(ver)
    op = DveOp(name, spec, subdim=False, uops_sha=sha)
    OPS.append(op)
    _dops.CUSTOM_DVE_SPECS[name] = spec
    _dops._SUB_OPCODE_FOR_NAME[name] = _dops._CUSTOM_DVE_ROW_BASE + len(OPS) - 1
    assert max(_dops._SUB_OPCODE_FOR_NAME.values()) < 0x20
    return op


def _dmq_ref(in0, in1, s0, s1, imm2):
    x = in0.astype(np.float32) * np.float32(s1)
    d = x - in1
    net = (d > s0).astype(np.float32) - (d < -s0).astype(np.float32)
    return in1 + net * s0


_d = Src0 * C1 - Src1
DM_STEP = _register(
    "DMQ_STEP_ANT",
    Spec(body=Src1 + ((_d > C0) - (_d < (Zero - C0))) * C0, reference=_dmq_ref),
)


# ------------------------------------------------------------ build program
def _build_program(th_val):
    nc = bacc.Bacc(None)
    xin = nc.dram_tensor("xin", [128, S * CW], I16, kind="ExternalInput")
    # rowgroup 0 spikes as fp8 recon-deltas; rowgroup 1 recon as fp16
    d8t = nc.dram_tensor("d8", [128, S * U], FP8, kind="ExternalOutput")
    spk = nc.dram_tensor("spk", [128, S * U], F16, kind="ExternalOutput")
    rwt = nc.dram_tensor("rw", [128, 2 * U], F32, kind="ExternalOutput")
    rlt = nc.dram_tensor("rl", [128, 2 * U], F32, kind="ExternalOutput")

    with tile.TileContext(nc) as tc:
        with (
            tc.tile_pool(name="xpin", bufs=1) as pinpool,
            tc.tile_pool(name="xring", bufs=RBUFS) as ringpool,
            tc.tile_pool(name="kp", bufs=KBUFS) as kpool,
            tc.tile_pool(name="sp", bufs=SBUFS) as spool,
            tc.tile_pool(name="dp", bufs=DBUFS) as dpool,
            tc.tile_pool(name="cp", bufs=1) as cpool,
        ):
            K0 = cpool.tile([128, 2 * U], F32)
            nc.gpsimd.memset(K0[:], 0.0)

            pin = []
            for p in range(NPIN):
                xp = pinpool.tile([128, PL * CW], I16, tag=f"pin{p}", name=f"xp{p}")
                if p == 0 and SPLIT_FIRST:
                    sf = SPLIT_FIRST
                    nc.sync.dma_start(xp[:, 0 : sf * CW], xin[:, 0 : sf * CW])
                    nc.sync.dma_start(
                        xp[:, sf * CW : PL * CW], xin[:, sf * CW : PL * CW]
                    )
                else:
                    nc.sync.dma_start(
                        xp[:], xin[:, p * PL * CW : (p + 1) * PL * CW]
                    )
                pin.append(xp)

            def in0_ap(xt, row, off):
                # [128, 2, U] view: 2 rowgroups, U lanes, group stride U+1
                g2 = xt[:, row * CW : (row + 1) * CW].rearrange(
                    "p (g c) -> p g c", g=2
                )
                return g2[:, :, off : off + U]

            kprev = K0[:]
            kprev_tile = None  # previous K piece (for Pool boundary diff)
            for pc in range(NPIECE):
                i0 = pc * PL
                if i0 < W:
                    xt, off = pin[pc], 0
                elif i0 < S:
                    xt = ringpool.tile([128, PL * CW], I16, tag="xr", name=f"xr{pc}")
                    nc.sync.dma_start(xt[:], xin[:, i0 * CW : (i0 + PL) * CW])
                    off = 0
                else:
                    xt, off = pin[pc - S // PL], 1

                KP = kpool.tile([128, PL * 2 * U], F32, tag="k", name=f"k{pc}")
                for il in range(PL):
                    nc.vector._custom_dve(
                        DM_STEP,
                        out=KP[:, il * 2 * U : (il + 1) * 2 * U],
                        in0=in0_ap(xt, il, off),
                        in1=kprev,
                        s0=float(th_val),
                        s1=float(QF),
                    )
                    kprev = KP[:, il * 2 * U : (il + 1) * 2 * U]

                if i0 + PL == W:
                    # recon entering emit span (step W-1)
                    nc.sync.dma_start(rwt[:], KP[:, (PL - 1) * 2 * U : PL * 2 * U])
                if i0 >= W and ABL_EXTRACT:
                    tl0 = i0 - W
                    ep = (i0 - W) // PL       # emit piece index
                    gsl = ep % OGRP           # slot within output group
                    kv = KP[:].rearrange("p (s l) -> p s l", s=PL)
                    if gsl == 0:
                        D8 = dpool.tile([128, OGRP * PL, U], FP8, tag="d", name=f"d{pc}")
                        SP = spool.tile([128, OGRP * PL, U], F16, tag="s", name=f"s{pc}")
                    r0 = gsl * PL
                    pv = kprev_tile[:].rearrange("p (s l) -> p s l", s=PL)
                    last = False  # tail-split regressed in TimelineSim; disabled
                    # last piece: extract in halves so the first half (and its
                    # DMA) overlaps the final DVE steps; tail = a 2-row chain
                    subs = ((0, PL // 2), (PL // 2, PL)) if last else ((0, PL),)
                    for a, b in subs:
                        # rowgroup 0: Pool diff -> fp8
                        if a == 0:
                            nc.gpsimd.tensor_tensor(
                                D8[:, r0 : r0 + 1, :],
                                kv[:, 0:1, 0:U],
                                pv[:, PL - 1 : PL, 0:U],
                                AluOpType.subtract,
                            )
                            lo = 1
                        else:
                            lo = a
                        nc.gpsimd.tensor_tensor(
                            D8[:, r0 + lo : r0 + b, :],
                            kv[:, lo:b, 0:U],
                            kv[:, lo - 1 : b - 1, 0:U],
                            AluOpType.subtract,
                        )
                        # rowgroup 1: recon as fp16 via ACT
                        nc.scalar.activation(
                            SP[:, r0 + a : r0 + b, :],
                            kv[:, a:b, U : 2 * U],
                            mybir.ActivationFunctionType.Copy,
                        )
                        if last:
                            nc.scalar.dma_start(
                                d8t[:, (tl0 + a) * U : (tl0 + b) * U],
                                D8[:, r0 + a : r0 + b, :],
                            )
                            nc.scalar.dma_start(
                                spk[:, (tl0 + a) * U : (tl0 + b) * U],
                                SP[:, r0 + a : r0 + b, :],
                            )
                    if not last and gsl == OGRP - 1:
                        n = (gsl + 1) * PL
                        g0 = tl0 - gsl * PL
                        nc.scalar.dma_start(
                            d8t[:, g0 * U : (g0 + n) * U], D8[:, 0:n, :]
                        )
                        nc.scalar.dma_start(
                            spk[:, g0 * U : (g0 + n) * U], SP[:, 0:n, :]
                        )
                if pc == NPIECE - 1:
                    nc.sync.dma_start(rlt[:], KP[:, (PL - 1) * 2 * U : PL * 2 * U])
                kprev_tile = KP
    nc.finalize()
    return nc


_NC_CACHE = {}


def _get_program(th_val):
    key = float(th_val)
    if key not in _NC_CACHE:
        _NC_CACHE[key] = _build_program(key)
    return _NC_CACHE[key]


# ------------------------------------------------------------ host helpers
def quantize(xs):
    """xs (R, T) f32 -> (int16 codes, dequantized f32 x-hat)."""
    k = np.clip(np.rint(xs * np.float32(2.0 ** QLOG)), -32767, 32767).astype(
        np.int16
    )
    return k, k.astype(np.float32) * QF


def build_xin(k_core):
    """k_core: (256, T) int16 -> xin (128, S*CW) int16.

    xin[p, i*CW + g*(U+1) + j] = kpad[g*128+p, j*S + i], kpad = k_core
    left-padded with W zeros (tail pad never consumed).
    """
    kpad = np.zeros((RPC, W + T + S), dtype=np.int16)
    kpad[:, W : W + T] = k_core
    st_r, st_e = kpad.strides
    A = np.lib.stride_tricks.as_strided(
        kpad, shape=(RPC, U + 1, S), strides=(st_r, S * st_e, st_e)
    )  # A[r, j, i] = kpad[r, j*S + i]
    out = np.empty((128, S, 2, U + 1), dtype=np.int16)
    At = A.transpose(0, 2, 1)  # (r, i, j)
    out[:, :, 0, :] = At[:128]
    out[:, :, 1, :] = At[128:]
    return np.ascontiguousarray(out.reshape(128, S * CW))


def decode_outputs(results, xq, th):
    """results: per-core dicts with 'd8' (fp8), 'spk' (fp16), 'rw','rl' (f32).
    xq: (R, T) f32 dequantized input. Returns the exact x-hat-scan spikes
    (R, T) f32."""
    th = np.float32(th)
    half = th / np.float32(2)
    out = np.empty((R, T), dtype=np.float32)
    rw = np.empty((R, U), dtype=np.float32)
    rl = np.empty((R, U), dtype=np.float32)
    for core in range(N_CORES):
        r = results[core]
        rw2 = np.asarray(r["rw"]).reshape(128, 2, U)
        # rowgroup 0: fp8 recon-deltas
        d8 = np.asarray(r["d8"]).reshape(128, S, U).astype(np.float32)
        s0 = (d8 > half).astype(np.float32) - (d8 < -half).astype(np.float32)
        # rowgroup 1: fp16 recon -> diff
        k16 = np.asarray(r["spk"]).reshape(128, S, U).astype(np.float32)
        d1 = np.empty_like(k16)
        d1[:, 0] = k16[:, 0] - rw2[:, 1]
        d1[:, 1:] = k16[:, 1:] - k16[:, :-1]
        s1 = (d1 > half).astype(np.float32) - (d1 < -half).astype(np.float32)
        blk = out[core * RPC : (core + 1) * RPC].reshape(2, 128, U, S)
        blk[0] = s0.transpose(0, 2, 1)
        blk[1] = s1.transpose(0, 2, 1)
        rw[core * RPC : (core + 1) * RPC] = rw2.transpose(1, 0, 2).reshape(RPC, U)
        rl[core * RPC : (core + 1) * RPC] = (
            np.asarray(r["rl"]).reshape(128, 2, U).transpose(1, 0, 2).reshape(RPC, U)
        )

    # ---- chain-verified fixup (see module docstring): sequential over
    # chunks (vectorized over rows), so cascaded breaks cost one pass.
    rlc = rl[:, 0].copy()  # corrected end state of the previous chunk
    outv = out.reshape(R, U, S)
    for j in range(1, U):
        bad = np.abs(rw[:, j] - rlc) > half
        if bad.any():
            rows = np.nonzero(bad)[0]
            xseg = xq[:, j * S : (j + 1) * S][rows]
            rcur = rlc[rows].copy()
            seg = np.empty((len(rows), S), dtype=np.float32)
            for i in range(S):
                dd = xseg[:, i] - rcur
                net = (dd > th).astype(np.float32) - (dd < -th).astype(np.float32)
                rcur = rcur + net * th
                seg[:, i] = net
            outv[rows, j] = seg
            rlc = rl[:, j].copy()
            rlc[rows] = rcur
        else:
            rlc = rl[:, j]
    return out


# ------------------------------------------------------------------- kernel
def kernel(x, threshold):
    x = np.ascontiguousarray(np.asarray(x, dtype=np.float32))
    th = np.float32(
        min(max(np.float32(threshold), np.float32(0.01)), np.float32(0.5))
    )
    assert x.shape == (B, C, T)

    xs = x.reshape(R, T)
    k, xq = quantize(xs)

    in_maps = []
    for core in range(N_CORES):
        xin = build_xin(k[core * RPC : (core + 1) * RPC])
        in_maps.append({"xin": xin})

    nc = _get_program(th)
    res = run_bass_kernel_spmd(nc, in_maps, list(range(N_CORES)))

    out = decode_outputs(res.results, xq, th)
    return out.reshape(B, C, T)


if __name__ == "__main__":
    rng = np.random.default_rng(0)
    xv = rng.normal(0, 1, (B, C, T)).astype(np.float32)
    o = kernel(x=xv, threshold=np.float32(0.1))
    print("kernel ran; out", o.shape, o.dtype, np.unique(o))


# revision 25
# speedup vs baseline: 1.1768x; 1.0489x over previous
"""Delta-modulation encoder on 8 Trainium2 NeuronCores.

The reference is a sequential scan over T: recon tracks x in steps of
+-th, spikes = step direction. Parallelization: rows (b,c) are sharded
256-per-core (2 rowgroups x 128 partitions); each rowgroup's time axis is
split into U chunks of S steps, each chunk warm-started W steps early from
recon=0 (the recurrence self-synchronizes: warm and true trajectories
differ by a multiple of th and coalesce). Chunk 0's window is zero-padded
on the left, which keeps recon at exactly 0 through warmup, so every chunk
runs identical code.

Per time-step the whole core does ONE fused DVE instruction of width
2U covering all lanes of both rowgroups:

    recon' = recon + ((xq*q - recon) > th)*th - ((xq*q - recon) < -th)*th

x is shipped as int16 fixed point (q = 2^-13, clamped to +-4): the scan's
decisions only flip when x falls within q/2 of a threshold boundary
(measured: ~1.7k flips over 33.5M elements, rel err 7e-3, vs the 2e-2
gate). q is a power of two so the dequantized grid is exact in f32 and
the hardware trajectory is bit-reproducible on the host.

Input is streamed deduplicated: step i of chunk j reads x[j*S - W + i],
and the host lays x out as stream[i, j] = xpad[j*S + i] with one padded
column per rowgroup, so warmup rows are re-read from SBUF (shifted one
lane) instead of re-transferred.

Spike extraction (off the DVE critical path): rowgroup 0's recon deltas
go through Pool (tensor_tensor subtract -> fp8, sign recovered exactly on
host); rowgroup 1's recon ships via ACT as fp16 (error < th/2 for any
th >= 0.01, host differences exactly).

Correctness equals the full x-hat scan for ANY W via a host-side chain
check: the kernel ships each lane's recon entering its emit span (rw) and
at window end (rl). Chunk j is provably on the x-hat trajectory iff rw[j]
matches the corrected rl[j-1] within th/2 (real warmup gaps are multiples
of th; coalesced-but-differently-rounded walkers differ by ulps); broken
lanes are recomputed on the host from the verified checkpoint.
"""

import sys

for _p in ("/opt/trn_rl_repo",):
    if _p not in sys.path:
        sys.path.insert(0, _p)

import ml_dtypes
import numpy as np

from concourse import bacc, mybir, tile
from concourse.bass_utils import run_bass_kernel_spmd
from concourse.dve_spec import Spec, Src0, Src1, C0, C1, Zero, lower
from concourse.dve_ops import DveOp, OPS
import concourse.dve_ops as _dops
from concourse.dve_uop import DveOpSpec
from concourse.mybir import AluOpType

# ---------------------------------------------------------------- constants
B, C, T = 32, 64, 16384
N_CORES = 8
R = B * C                 # 2048 rows
RPC = R // N_CORES        # 256 rows per core (2 rowgroups x 128 partitions)
U = 256                   # time chunks per rowgroup
S = T // U                # emitted steps per chunk
W = 16                    # warmup steps
L = W + S                 # processed steps per chunk
PL = 4                    # steps per piece (DMA/extraction granularity)
CW = 2 * U + 2            # stream row width: 2 rowgroups x (U + 1 pad col)
NPIN = W // PL            # pinned x pieces (re-read at steps >= S)
NPIECE = L // PL
RBUFS = 8                 # x ring buffers
KBUFS = 10                # K piece buffers
SBUFS = 4                 # fp16 out staging buffers
DBUFS = 4                 # fp8 out staging buffers
SPLIT_FIRST = 2           # rows of pin piece 0 shipped in a separate first DMA
OGRP = 1                  # emit pieces per output DMA
ABL_EXTRACT = True        # ablation: emit extraction + out DMA
ABL_POOL = True           # ablation: use Pool fp8 path for rowgroup 0
QLOG = 13
QF = np.float32(2.0 ** -QLOG)
F32 = mybir.dt.float32
F16 = mybir.dt.float16
FP8 = mybir.dt.float8e4
I16 = mybir.dt.int16
assert W % PL == 0 and S % PL == 0 and W <= S and U * S == T


# ------------------------------------------------------- custom DVE op def
def _register(name, spec):
    sha = {}
    for ver in ("v3", "v4"):
        sha[ver] = DveOpSpec(
            name=name, opcode=0, uops=lower(spec, ver=ver), rd1_en=True
        ).sha(ver)
    op = DveOp(name, spec, subdim=False, uops_sha=sha)
    OPS.append(op)
    _dops.CUSTOM_DVE_SPECS[name] = spec
    _dops._SUB_OPCODE_FOR_NAME[name] = _dops._CUSTOM_DVE_ROW_BASE + len(OPS) - 1
    assert max(_dops._SUB_OPCODE_FOR_NAME.values()) < 0x20
    return op


def _dmq_ref(in0, in1, s0, s1, imm2):
    x = in0.astype(np.float32) * np.float32(s1)
    d = x - in1
    net = (d > s0).astype(np.float32) - (d < -s0).astype(np.float32)
    return in1 + net * s0


_d = Src0 * C1 - Src1
DM_STEP = _register(
    "DMQ_STEP_ANT",
    Spec(body=Src1 + ((_d > C0) - (_d < (Zero - C0))) * C0, reference=_dmq_ref),
)


# ------------------------------------------------------------ build program
def _build_program(th_val):
    nc = bacc.Bacc(None)
    xin = nc.dram_tensor("xin", [128, S * CW], I16, kind="ExternalInput")
    # rowgroup 0 spikes as fp8 recon-deltas; rowgroup 1 recon as fp16
    d8t = nc.dram_tensor("d8", [128, S * U], FP8, kind="ExternalOutput")
    spk = nc.dram_tensor("spk", [128, S * U], F16, kind="ExternalOutput")
    rwt = nc.dram_tensor("rw", [128, 2 * U], F32, kind="ExternalOutput")
    rlt = nc.dram_tensor("rl", [128, 2 * U], F32, kind="ExternalOutput")

    with tile.TileContext(nc) as tc:
        with (
            tc.tile_pool(name="xpin", bufs=1) as pinpool,
            tc.tile_pool(name="xring", bufs=RBUFS) as ringpool,
            tc.tile_pool(name="kp", bufs=KBUFS) as kpool,
            tc.tile_pool(name="sp", bufs=SBUFS) as spool,
            tc.tile_pool(name="dp", bufs=DBUFS) as dpool,
            tc.tile_pool(name="cp", bufs=1) as cpool,
        ):
            K0 = cpool.tile([128, 2 * U], F32)
            nc.gpsimd.memset(K0[:], 0.0)

            pin = []
            for p in range(NPIN):
                xp = pinpool.tile([128, PL * CW], I16, tag=f"pin{p}", name=f"xp{p}")
                if p == 0 and SPLIT_FIRST:
                    sf = SPLIT_FIRST
                    nc.sync.dma_start(xp[:, 0 : sf * CW], xin[:, 0 : sf * CW])
                    nc.sync.dma_start(
                        xp[:, sf * CW : PL * CW], xin[:, sf * CW : PL * CW]
                    )
                else:
                    nc.sync.dma_start(
                        xp[:], xin[:, p * PL * CW : (p + 1) * PL * CW]
                    )
                pin.append(xp)

            def in0_ap(xt, row, g, off):
                # rowgroup g's U lanes of stream row `row` (contiguous)
                base = row * CW + g * (U + 1) + off
                return xt[:, base : base + U]

            # two interleaved chains (one per rowgroup): each op's true
            # dependency is 2 instructions back, so Tile's completion
            # semaphore (engine + drain + sem-prop, ~95ns) hides under the
            # other chain's engine time instead of serializing every step.
            kprev = [K0[:, 0:U], K0[:, U : 2 * U]]
            kprev_tile = None  # previous K piece (for Pool boundary diff)
            for pc in range(NPIECE):
                i0 = pc * PL
                if i0 < W:
                    xt, off = pin[pc], 0
                elif i0 < S:
                    xt = ringpool.tile([128, PL * CW], I16, tag="xr", name=f"xr{pc}")
                    nc.sync.dma_start(xt[:], xin[:, i0 * CW : (i0 + PL) * CW])
                    off = 0
                else:
                    xt, off = pin[pc - S // PL], 1

                KP = kpool.tile([128, PL * 2 * U], F32, tag="k", name=f"k{pc}")
                for il in range(PL):
                    for g in (0, 1):
                        o0 = il * 2 * U + g * U
                        nc.vector._custom_dve(
                            DM_STEP,
                            out=KP[:, o0 : o0 + U],
                            in0=in0_ap(xt, il, g, off),
                            in1=kprev[g],
                            s0=float(th_val),
                            s1=float(QF),
                        )
                        kprev[g] = KP[:, o0 : o0 + U]

                if i0 + PL == W:
                    # recon entering emit span (step W-1)
                    nc.sync.dma_start(rwt[:], KP[:, (PL - 1) * 2 * U : PL * 2 * U])
                if i0 >= W and ABL_EXTRACT:
                    tl0 = i0 - W
                    ep = (i0 - W) // PL       # emit piece index
                    gsl = ep % OGRP           # slot within output group
                    kv = KP[:].rearrange("p (s l) -> p s l", s=PL)
                    if gsl == 0:
                        D8 = dpool.tile([128, OGRP * PL, U], FP8, tag="d", name=f"d{pc}")
                        SP = spool.tile([128, OGRP * PL, U], F16, tag="s", name=f"s{pc}")
                    r0 = gsl * PL
                    pv = kprev_tile[:].rearrange("p (s l) -> p s l", s=PL)
                    last = False  # tail-split regressed in TimelineSim; disabled
                    # last piece: extract in halves so the first half (and its
                    # DMA) overlaps the final DVE steps; tail = a 2-row chain
                    subs = ((0, PL // 2), (PL // 2, PL)) if last else ((0, PL),)
                    for a, b in subs:
                        # rowgroup 0: Pool diff -> fp8
                        if a == 0:
                            nc.gpsimd.tensor_tensor(
                                D8[:, r0 : r0 + 1, :],
                                kv[:, 0:1, 0:U],
                                pv[:, PL - 1 : PL, 0:U],
                                AluOpType.subtract,
                            )
                            lo = 1
                        else:
                            lo = a
                        nc.gpsimd.tensor_tensor(
                            D8[:, r0 + lo : r0 + b, :],
                            kv[:, lo:b, 0:U],
                            kv[:, lo - 1 : b - 1, 0:U],
                            AluOpType.subtract,
                        )
                        # rowgroup 1: recon as fp16 via ACT
                        nc.scalar.activation(
                            SP[:, r0 + a : r0 + b, :],
                            kv[:, a:b, U : 2 * U],
                            mybir.ActivationFunctionType.Copy,
                        )
                        if last:
                            nc.scalar.dma_start(
                                d8t[:, (tl0 + a) * U : (tl0 + b) * U],
                                D8[:, r0 + a : r0 + b, :],
                            )
                            nc.scalar.dma_start(
                                spk[:, (tl0 + a) * U : (tl0 + b) * U],
                                SP[:, r0 + a : r0 + b, :],
                            )
                    if not last and gsl == OGRP - 1:
                        n = (gsl + 1) * PL
                        g0 = tl0 - gsl * PL
                        nc.scalar.dma_start(
                            d8t[:, g0 * U : (g0 + n) * U], D8[:, 0:n, :]
                        )
                        nc.scalar.dma_start(
                            spk[:, g0 * U : (g0 + n) * U], SP[:, 0:n, :]
                        )
                if pc == NPIECE - 1:
                    nc.sync.dma_start(rlt[:], KP[:, (PL - 1) * 2 * U : PL * 2 * U])
                kprev_tile = KP
    nc.finalize()
    return nc


_NC_CACHE = {}


def _get_program(th_val):
    key = float(th_val)
    if key not in _NC_CACHE:
        _NC_CACHE[key] = _build_program(key)
    return _NC_CACHE[key]


# ------------------------------------------------------------ host helpers
def quantize(xs):
    """xs (R, T) f32 -> (int16 codes, dequantized f32 x-hat)."""
    k = np.clip(np.rint(xs * np.float32(2.0 ** QLOG)), -32767, 32767).astype(
        np.int16
    )
    return k, k.astype(np.float32) * QF


def build_xin(k_core):
    """k_core: (256, T) int16 -> xin (128, S*CW) int16.

    xin[p, i*CW + g*(U+1) + j] = kpad[g*128+p, j*S + i], kpad = k_core
    left-padded with W zeros (tail pad never consumed).
    """
    kpad = np.zeros((RPC, W + T + S), dtype=np.int16)
    kpad[:, W : W + T] = k_core
    st_r, st_e = kpad.strides
    A = np.lib.stride_tricks.as_strided(
        kpad, shape=(RPC, U + 1, S), strides=(st_r, S * st_e, st_e)
    )  # A[r, j, i] = kpad[r, j*S + i]
    out = np.empty((128, S, 2, U + 1), dtype=np.int16)
    At = A.transpose(0, 2, 1)  # (r, i, j)
    out[:, :, 0, :] = At[:128]
    out[:, :, 1, :] = At[128:]
    return np.ascontiguousarray(out.reshape(128, S * CW))


def decode_outputs(results, xq, th):
    """results: per-core dicts with 'd8' (fp8), 'spk' (fp16), 'rw','rl' (f32).
    xq: (R, T) f32 dequantized input. Returns the exact x-hat-scan spikes
    (R, T) f32."""
    th = np.float32(th)
    half = th / np.float32(2)
    out = np.empty((R, T), dtype=np.float32)
    rw = np.empty((R, U), dtype=np.float32)
    rl = np.empty((R, U), dtype=np.float32)
    for core in range(N_CORES):
        r = results[core]
        rw2 = np.asarray(r["rw"]).reshape(128, 2, U)
        # rowgroup 0: fp8 recon-deltas
        d8 = np.asarray(r["d8"]).reshape(128, S, U).astype(np.float32)
        s0 = (d8 > half).astype(np.float32) - (d8 < -half).astype(np.float32)
        # rowgroup 1: fp16 recon -> diff
        k16 = np.asarray(r["spk"]).reshape(128, S, U).astype(np.float32)
        d1 = np.empty_like(k16)
        d1[:, 0] = k16[:, 0] - rw2[:, 1]
        d1[:, 1:] = k16[:, 1:] - k16[:, :-1]
        s1 = (d1 > half).astype(np.float32) - (d1 < -half).astype(np.float32)
        blk = out[core * RPC : (core + 1) * RPC].reshape(2, 128, U, S)
        blk[0] = s0.transpose(0, 2, 1)
        blk[1] = s1.transpose(0, 2, 1)
        rw[core * RPC : (core + 1) * RPC] = rw2.transpose(1, 0, 2).reshape(RPC, U)
        rl[core * RPC : (core + 1) * RPC] = (
            np.asarray(r["rl"]).reshape(128, 2, U).transpose(1, 0, 2).reshape(RPC, U)
        )

    # ---- chain-verified fixup (see module docstring): sequential over
    # chunks (vectorized over rows), so cascaded breaks cost one pass.
    rlc = rl[:, 0].copy()  # corrected end state of the previous chunk
    outv = out.reshape(R, U, S)
    for j in range(1, U):
        bad = np.abs(rw[:, j] - rlc) > half
        if bad.any():
            rows = np.nonzero(bad)[0]
            xseg = xq[:, j * S : (j + 1) * S][rows]
            rcur = rlc[rows].copy()
            seg = np.empty((len(rows), S), dtype=np.float32)
            for i in range(S):
                dd = xseg[:, i] - rcur
                net = (dd > th).astype(np.float32) - (dd < -th).astype(np.float32)
                rcur = rcur + net * th
                seg[:, i] = net
            outv[rows, j] = seg
            rlc = rl[:, j].copy()
            rlc[rows] = rcur
        else:
            rlc = rl[:, j]
    return out


# ------------------------------------------------------------------- kernel
def kernel(x, threshold):
    x = np.ascontiguousarray(np.asarray(x, dtype=np.float32))
    th = np.float32(
        min(max(np.float32(threshold), np.float32(0.01)), np.float32(0.5))
    )
    assert x.shape == (B, C, T)

    xs = x.reshape(R, T)
    k, xq = quantize(xs)

    in_maps = []
    for core in range(N_CORES):
        xin = build_xin(k[core * RPC : (core + 1) * RPC])
        in_maps.append({"xin": xin})

    nc = _get_program(th)
    res = run_bass_kernel_spmd(nc, in_maps, list(range(N_CORES)))

    out = decode_outputs(res.results, xq, th)
    return out.reshape(B, C, T)


if __name__ == "__main__":
    rng = np.random.default_rng(0)
    xv = rng.normal(0, 1, (B, C, T)).astype(np.float32)
    o = kernel(x=xv, threshold=np.float32(0.1))
    print("kernel ran; out", o.shape, o.dtype, np.unique(o))


# revision 26
# speedup vs baseline: 1.1828x; 1.0050x over previous
"""Delta-modulation encoder on 8 Trainium2 NeuronCores.

The reference is a sequential scan over T: recon tracks x in steps of
+-th, spikes = step direction. Parallelization: rows (b,c) are sharded
256-per-core (2 rowgroups x 128 partitions); each rowgroup's time axis is
split into U chunks of S steps, each chunk warm-started W steps early from
recon=0 (the recurrence self-synchronizes: warm and true trajectories
differ by a multiple of th and coalesce). Chunk 0's window is zero-padded
on the left, which keeps recon at exactly 0 through warmup, so every chunk
runs identical code.

Per time-step the whole core does ONE fused DVE instruction of width
2U covering all lanes of both rowgroups:

    recon' = recon + ((xq*q - recon) > th)*th - ((xq*q - recon) < -th)*th

x is shipped as int16 fixed point (q = 2^-13, clamped to +-4): the scan's
decisions only flip when x falls within q/2 of a threshold boundary
(measured: ~1.7k flips over 33.5M elements, rel err 7e-3, vs the 2e-2
gate). q is a power of two so the dequantized grid is exact in f32 and
the hardware trajectory is bit-reproducible on the host.

Input is streamed deduplicated: step i of chunk j reads x[j*S - W + i],
and the host lays x out as stream[i, j] = xpad[j*S + i] with one padded
column per rowgroup, so warmup rows are re-read from SBUF (shifted one
lane) instead of re-transferred.

Spike extraction (off the DVE critical path): rowgroup 0's recon deltas
go through Pool (tensor_tensor subtract -> fp8, sign recovered exactly on
host); rowgroup 1's recon ships via ACT as fp16 (error < th/2 for any
th >= 0.01, host differences exactly).

Correctness equals the full x-hat scan for ANY W via a host-side chain
check: the kernel ships each lane's recon entering its emit span (rw) and
at window end (rl). Chunk j is provably on the x-hat trajectory iff rw[j]
matches the corrected rl[j-1] within th/2 (real warmup gaps are multiples
of th; coalesced-but-differently-rounded walkers differ by ulps); broken
lanes are recomputed on the host from the verified checkpoint.
"""

import sys

for _p in ("/opt/trn_rl_repo",):
    if _p not in sys.path:
        sys.path.insert(0, _p)

import ml_dtypes
import numpy as np

from concourse import bacc, mybir, tile
from concourse.bass_utils import run_bass_kernel_spmd
from concourse.dve_spec import Spec, Src0, Src1, C0, C1, Zero, lower
from concourse.dve_ops import DveOp, OPS
import concourse.dve_ops as _dops
from concourse.dve_uop import DveOpSpec
from concourse.mybir import AluOpType

# ---------------------------------------------------------------- constants
B, C, T = 32, 64, 16384
N_CORES = 8
R = B * C                 # 2048 rows
RPC = R // N_CORES        # 256 rows per core (2 rowgroups x 128 partitions)
U = 256                   # time chunks per rowgroup
S = T // U                # emitted steps per chunk
W = 16                    # warmup steps
L = W + S                 # processed steps per chunk
PL = 4                    # steps per piece (DMA/extraction granularity)
CW = 2 * U + 2            # stream row width: 2 rowgroups x (U + 1 pad col)
NPIN = W // PL            # pinned x pieces (re-read at steps >= S)
NPIECE = L // PL
RBUFS = 8                 # x ring buffers
KBUFS = 10                # K piece buffers
SBUFS = 4                 # fp16 out staging buffers
DBUFS = 4                 # fp8 out staging buffers
SPLIT_FIRST = 2           # rows of pin piece 0 shipped in a separate first DMA
OGRP = 1                  # emit pieces per output DMA
ABL_EXTRACT = True        # ablation: emit extraction + out DMA
ABL_POOL = True           # ablation: use Pool fp8 path for rowgroup 0
QLOG = 13
QF = np.float32(2.0 ** -QLOG)
F32 = mybir.dt.float32
F16 = mybir.dt.float16
FP8 = mybir.dt.float8e4
I16 = mybir.dt.int16
assert W % PL == 0 and S % PL == 0 and W <= S and U * S == T


# ------------------------------------------------------- custom DVE op def
def _register(name, spec):
    sha = {}
    for ver in ("v3", "v4"):
        sha[ver] = DveOpSpec(
            name=name, opcode=0, uops=lower(spec, ver=ver), rd1_en=True
        ).sha(ver)
    op = DveOp(name, spec, subdim=False, uops_sha=sha)
    OPS.append(op)
    _dops.CUSTOM_DVE_SPECS[name] = spec
    _dops._SUB_OPCODE_FOR_NAME[name] = _dops._CUSTOM_DVE_ROW_BASE + len(OPS) - 1
    assert max(_dops._SUB_OPCODE_FOR_NAME.values()) < 0x20
    return op


def _dmq_ref(in0, in1, s0, s1, imm2):
    x = in0.astype(np.float32) * np.float32(s1)
    d = x - in1
    net = (d > s0).astype(np.float32) - (d < -s0).astype(np.float32)
    return in1 + net * s0


_d = Src0 * C1 - Src1
DM_STEP = _register(
    "DMQ_STEP_ANT",
    Spec(body=Src1 + ((_d > C0) - (_d < (Zero - C0))) * C0, reference=_dmq_ref),
)


# ------------------------------------------------------------ build program
def _build_program(th_val):
    nc = bacc.Bacc(None)
    xin = nc.dram_tensor("xin", [128, S * CW], I16, kind="ExternalInput")
    # rowgroup 0 spikes as fp8 recon-deltas; rowgroup 1 recon as fp16
    d8t = nc.dram_tensor("d8", [128, S * U], FP8, kind="ExternalOutput")
    spk = nc.dram_tensor("spk", [128, S * U], F16, kind="ExternalOutput")
    rwt = nc.dram_tensor("rw", [128, 2 * U], F32, kind="ExternalOutput")
    rlt = nc.dram_tensor("rl", [128, 2 * U], F32, kind="ExternalOutput")

    with tile.TileContext(nc) as tc:
        with (
            tc.tile_pool(name="xpin", bufs=1) as pinpool,
            tc.tile_pool(name="xring", bufs=RBUFS) as ringpool,
            tc.tile_pool(name="kp", bufs=KBUFS) as kpool,
            tc.tile_pool(name="sp", bufs=SBUFS) as spool,
            tc.tile_pool(name="dp", bufs=DBUFS) as dpool,
            tc.tile_pool(name="cp", bufs=1) as cpool,
        ):
            K0 = cpool.tile([128, 2 * U], F32)
            nc.gpsimd.memset(K0[:], 0.0)

            pin = []
            for p in range(NPIN):
                xp = pinpool.tile([128, PL * CW], I16, tag=f"pin{p}", name=f"xp{p}")
                if p == 0 and SPLIT_FIRST:
                    sf = SPLIT_FIRST
                    nc.sync.dma_start(xp[:, 0 : sf * CW], xin[:, 0 : sf * CW])
                    nc.sync.dma_start(
                        xp[:, sf * CW : PL * CW], xin[:, sf * CW : PL * CW]
                    )
                else:
                    nc.sync.dma_start(
                        xp[:], xin[:, p * PL * CW : (p + 1) * PL * CW]
                    )
                pin.append(xp)

            def in0_ap(xt, row, g, off):
                # rowgroup g's U lanes of stream row `row` (contiguous)
                base = row * CW + g * (U + 1) + off
                return xt[:, base : base + U]

            # two interleaved chains (one per rowgroup): each op's true
            # dependency is 2 instructions back, so Tile's completion
            # semaphore (engine + drain + sem-prop, ~95ns) hides under the
            # other chain's engine time instead of serializing every step.
            kprev = [K0[:, 0:U], K0[:, U : 2 * U]]
            kprev_tile = None  # previous K piece (for Pool boundary diff)
            for pc in range(NPIECE):
                i0 = pc * PL
                if i0 < W:
                    xt, off = pin[pc], 0
                elif i0 < S:
                    xt = ringpool.tile([128, PL * CW], I16, tag="xr", name=f"xr{pc}")
                    nc.sync.dma_start(xt[:], xin[:, i0 * CW : (i0 + PL) * CW])
                    off = 0
                else:
                    xt, off = pin[pc - S // PL], 1

                KP = kpool.tile([128, PL * 2 * U], F32, tag="k", name=f"k{pc}")
                for il in range(PL):
                    for g in (0, 1):
                        o0 = il * 2 * U + g * U
                        nc.vector._custom_dve(
                            DM_STEP,
                            out=KP[:, o0 : o0 + U],
                            in0=in0_ap(xt, il, g, off),
                            in1=kprev[g],
                            s0=float(th_val),
                            s1=float(QF),
                        )
                        kprev[g] = KP[:, o0 : o0 + U]

                if i0 + PL == W:
                    # recon entering emit span (step W-1)
                    nc.sync.dma_start(rwt[:], KP[:, (PL - 1) * 2 * U : PL * 2 * U])
                if i0 >= W and ABL_EXTRACT:
                    tl0 = i0 - W
                    ep = (i0 - W) // PL       # emit piece index
                    gsl = ep % OGRP           # slot within output group
                    kv = KP[:].rearrange("p (s l) -> p s l", s=PL)
                    if gsl == 0:
                        D8 = dpool.tile([128, OGRP * PL, U], FP8, tag="d", name=f"d{pc}")
                        SP = spool.tile([128, OGRP * PL, U], F16, tag="s", name=f"s{pc}")
                    r0 = gsl * PL
                    pv = kprev_tile[:].rearrange("p (s l) -> p s l", s=PL)
                    last = False  # tail-split regressed in TimelineSim; disabled
                    # last piece: extract in halves so the first half (and its
                    # DMA) overlaps the final DVE steps; tail = a 2-row chain
                    subs = ((0, PL // 2), (PL // 2, PL)) if last else ((0, PL),)
                    # last piece: diff on the then-idle DVE for a shorter tail
                    deng = nc.vector if pc == NPIECE - 1 else nc.gpsimd
                    for a, b in subs:
                        # rowgroup 0: diff -> fp8
                        if a == 0:
                            deng.tensor_tensor(
                                D8[:, r0 : r0 + 1, :],
                                kv[:, 0:1, 0:U],
                                pv[:, PL - 1 : PL, 0:U],
                                AluOpType.subtract,
                            )
                            lo = 1
                        else:
                            lo = a
                        deng.tensor_tensor(
                            D8[:, r0 + lo : r0 + b, :],
                            kv[:, lo:b, 0:U],
                            kv[:, lo - 1 : b - 1, 0:U],
                            AluOpType.subtract,
                        )
                        # rowgroup 1: recon as fp16 via ACT
                        nc.scalar.activation(
                            SP[:, r0 + a : r0 + b, :],
                            kv[:, a:b, U : 2 * U],
                            mybir.ActivationFunctionType.Copy,
                        )
                        if last:
                            nc.scalar.dma_start(
                                d8t[:, (tl0 + a) * U : (tl0 + b) * U],
                                D8[:, r0 + a : r0 + b, :],
                            )
                            nc.scalar.dma_start(
                                spk[:, (tl0 + a) * U : (tl0 + b) * U],
                                SP[:, r0 + a : r0 + b, :],
                            )
                    if not last and gsl == OGRP - 1:
                        n = (gsl + 1) * PL
                        g0 = tl0 - gsl * PL
                        nc.scalar.dma_start(
                            d8t[:, g0 * U : (g0 + n) * U], D8[:, 0:n, :]
                        )
                        nc.scalar.dma_start(
                            spk[:, g0 * U : (g0 + n) * U], SP[:, 0:n, :]
                        )
                if pc == NPIECE - 1:
                    nc.sync.dma_start(rlt[:], KP[:, (PL - 1) * 2 * U : PL * 2 * U])
                kprev_tile = KP
    nc.finalize()
    return nc


_NC_CACHE = {}


def _get_program(th_val):
    key = float(th_val)
    if key not in _NC_CACHE:
        _NC_CACHE[key] = _build_program(key)
    return _NC_CACHE[key]


# ------------------------------------------------------------ host helpers
def quantize(xs):
    """xs (R, T) f32 -> (int16 codes, dequantized f32 x-hat)."""
    k = np.clip(np.rint(xs * np.float32(2.0 ** QLOG)), -32767, 32767).astype(
        np.int16
    )
    return k, k.astype(np.float32) * QF


def build_xin(k_core):
    """k_core: (256, T) int16 -> xin (128, S*CW) int16.

    xin[p, i*CW + g*(U+1) + j] = kpad[g*128+p, j*S + i], kpad = k_core
    left-padded with W zeros (tail pad never consumed).
    """
    kpad = np.zeros((RPC, W + T + S), dtype=np.int16)
    kpad[:, W : W + T] = k_core
    st_r, st_e = kpad.strides
    A = np.lib.stride_tricks.as_strided(
        kpad, shape=(RPC, U + 1, S), strides=(st_r, S * st_e, st_e)
    )  # A[r, j, i] = kpad[r, j*S + i]
    out = np.empty((128, S, 2, U + 1), dtype=np.int16)
    At = A.transpose(0, 2, 1)  # (r, i, j)
    out[:, :, 0, :] = At[:128]
    out[:, :, 1, :] = At[128:]
    return np.ascontiguousarray(out.reshape(128, S * CW))


def decode_outputs(results, xq, th):
    """results: per-core dicts with 'd8' (fp8), 'spk' (fp16), 'rw','rl' (f32).
    xq: (R, T) f32 dequantized input. Returns the exact x-hat-scan spikes
    (R, T) f32."""
    th = np.float32(th)
    half = th / np.float32(2)
    out = np.empty((R, T), dtype=np.float32)
    rw = np.empty((R, U), dtype=np.float32)
    rl = np.empty((R, U), dtype=np.float32)
    for core in range(N_CORES):
        r = results[core]
        rw2 = np.asarray(r["rw"]).reshape(128, 2, U)
        # rowgroup 0: fp8 recon-deltas
        d8 = np.asarray(r["d8"]).reshape(128, S, U).astype(np.float32)
        s0 = (d8 > half).astype(np.float32) - (d8 < -half).astype(np.float32)
        # rowgroup 1: fp16 recon -> diff
        k16 = np.asarray(r["spk"]).reshape(128, S, U).astype(np.float32)
        d1 = np.empty_like(k16)
        d1[:, 0] = k16[:, 0] - rw2[:, 1]
        d1[:, 1:] = k16[:, 1:] - k16[:, :-1]
        s1 = (d1 > half).astype(np.float32) - (d1 < -half).astype(np.float32)
        blk = out[core * RPC : (core + 1) * RPC].reshape(2, 128, U, S)
        blk[0] = s0.transpose(0, 2, 1)
        blk[1] = s1.transpose(0, 2, 1)
        rw[core * RPC : (core + 1) * RPC] = rw2.transpose(1, 0, 2).reshape(RPC, U)
        rl[core * RPC : (core + 1) * RPC] = (
            np.asarray(r["rl"]).reshape(128, 2, U).transpose(1, 0, 2).reshape(RPC, U)
        )

    # ---- chain-verified fixup (see module docstring): sequential over
    # chunks (vectorized over rows), so cascaded breaks cost one pass.
    rlc = rl[:, 0].copy()  # corrected end state of the previous chunk
    outv = out.reshape(R, U, S)
    for j in range(1, U):
        bad = np.abs(rw[:, j] - rlc) > half
        if bad.any():
            rows = np.nonzero(bad)[0]
            xseg = xq[:, j * S : (j + 1) * S][rows]
            rcur = rlc[rows].copy()
            seg = np.empty((len(rows), S), dtype=np.float32)
            for i in range(S):
                dd = xseg[:, i] - rcur
                net = (dd > th).astype(np.float32) - (dd < -th).astype(np.float32)
                rcur = rcur + net * th
                seg[:, i] = net
            outv[rows, j] = seg
            rlc = rl[:, j].copy()
            rlc[rows] = rcur
        else:
            rlc = rl[:, j]
    return out


# ------------------------------------------------------------------- kernel
def kernel(x, threshold):
    x = np.ascontiguousarray(np.asarray(x, dtype=np.float32))
    th = np.float32(
        min(max(np.float32(threshold), np.float32(0.01)), np.float32(0.5))
    )
    assert x.shape == (B, C, T)

    xs = x.reshape(R, T)
    k, xq = quantize(xs)

    in_maps = []
    for core in range(N_CORES):
        xin = build_xin(k[core * RPC : (core + 1) * RPC])
        in_maps.append({"xin": xin})

    nc = _get_program(th)
    res = run_bass_kernel_spmd(nc, in_maps, list(range(N_CORES)))

    out = decode_outputs(res.results, xq, th)
    return out.reshape(B, C, T)


if __name__ == "__main__":
    rng = np.random.default_rng(0)
    xv = rng.normal(0, 1, (B, C, T)).astype(np.float32)
    o = kernel(x=xv, threshold=np.float32(0.1))
    print("kernel ran; out", o.shape, o.dtype, np.unique(o))


# revision 30
# speedup vs baseline: 1.1852x; 1.0021x over previous
"""Delta-modulation encoder on 8 Trainium2 NeuronCores.

The reference is a sequential scan over T: recon tracks x in steps of
+-th, spikes = step direction. Parallelization: rows (b,c) are sharded
256-per-core (2 rowgroups x 128 partitions); each rowgroup's time axis is
split into U chunks of S steps, each chunk warm-started W steps early from
recon=0 (the recurrence self-synchronizes: warm and true trajectories
differ by a multiple of th and coalesce). Chunk 0's window is zero-padded
on the left, which keeps recon at exactly 0 through warmup, so every chunk
runs identical code.

Per time-step the whole core does ONE fused DVE instruction of width
2U covering all lanes of both rowgroups:

    recon' = recon + ((xq*q - recon) > th)*th - ((xq*q - recon) < -th)*th

x is shipped as int16 fixed point (q = 2^-13, clamped to +-4): the scan's
decisions only flip when x falls within q/2 of a threshold boundary
(measured: ~1.7k flips over 33.5M elements, rel err 7e-3, vs the 2e-2
gate). q is a power of two so the dequantized grid is exact in f32 and
the hardware trajectory is bit-reproducible on the host.

Input is streamed deduplicated: step i of chunk j reads x[j*S - W + i],
and the host lays x out as stream[i, j] = xpad[j*S + i] with one padded
column per rowgroup, so warmup rows are re-read from SBUF (shifted one
lane) instead of re-transferred.

Spike extraction (off the DVE critical path): rowgroup 0's recon deltas
go through Pool (tensor_tensor subtract -> fp8, sign recovered exactly on
host); rowgroup 1's recon ships via ACT as fp16 (error < th/2 for any
th >= 0.01, host differences exactly).

Correctness equals the full x-hat scan for ANY W via a host-side chain
check: the kernel ships each lane's recon entering its emit span (rw) and
at window end (rl). Chunk j is provably on the x-hat trajectory iff rw[j]
matches the corrected rl[j-1] within th/2 (real warmup gaps are multiples
of th; coalesced-but-differently-rounded walkers differ by ulps); broken
lanes are recomputed on the host from the verified checkpoint.
"""

import sys

for _p in ("/opt/trn_rl_repo",):
    if _p not in sys.path:
        sys.path.insert(0, _p)

import ml_dtypes
import numpy as np

from concourse import bacc, mybir, tile
from concourse.bass_utils import run_bass_kernel_spmd
from concourse.dve_spec import Spec, Src0, Src1, C0, C1, Zero, lower
from concourse.dve_ops import DveOp, OPS
import concourse.dve_ops as _dops
from concourse.dve_uop import DveOpSpec
from concourse.mybir import AluOpType

# ---------------------------------------------------------------- constants
B, C, T = 32, 64, 16384
N_CORES = 8
R = B * C                 # 2048 rows
RPC = R // N_CORES        # 256 rows per core (2 rowgroups x 128 partitions)
U = 256                   # time chunks per rowgroup
S = T // U                # emitted steps per chunk
W = 16                    # warmup steps
L = W + S                 # processed steps per chunk
PL = 4                    # steps per piece (DMA/extraction granularity)
CW = 2 * U + 2            # stream row width: 2 rowgroups x (U + 1 pad col)
NPIN = W // PL            # pinned x pieces (re-read at steps >= S)
NPIECE = L // PL
RBUFS = 8                 # x ring buffers
KBUFS = 10                # K piece buffers
SBUFS = 4                 # fp16 out staging buffers
DBUFS = 4                 # fp8 out staging buffers
SPLIT_FIRST = 2           # rows of pin piece 0 shipped in a separate first DMA
OGRP = 1                  # emit pieces per output DMA
ABL_EXTRACT = True        # ablation: emit extraction + out DMA
ABL_POOL = True           # ablation: use Pool fp8 path for rowgroup 0
QLOG = 13
QF = np.float32(2.0 ** -QLOG)
F32 = mybir.dt.float32
F16 = mybir.dt.float16
FP8 = mybir.dt.float8e4
I16 = mybir.dt.int16
assert W % PL == 0 and S % PL == 0 and W <= S and U * S == T


# ------------------------------------------------------- custom DVE op def
def _register(name, spec):
    sha = {}
    for ver in ("v3", "v4"):
        sha[ver] = DveOpSpec(
            name=name, opcode=0, uops=lower(spec, ver=ver), rd1_en=True
        ).sha(ver)
    op = DveOp(name, spec, subdim=False, uops_sha=sha)
    OPS.append(op)
    _dops.CUSTOM_DVE_SPECS[name] = spec
    _dops._SUB_OPCODE_FOR_NAME[name] = _dops._CUSTOM_DVE_ROW_BASE + len(OPS) - 1
    assert max(_dops._SUB_OPCODE_FOR_NAME.values()) < 0x20
    return op


def _dmq_ref(in0, in1, s0, s1, imm2):
    x = in0.astype(np.float32) * np.float32(s1)
    d = x - in1
    net = (d > s0).astype(np.float32) - (d < -s0).astype(np.float32)
    return in1 + net * s0


_d = Src0 * C1 - Src1
DM_STEP = _register(
    "DMQ_STEP_ANT",
    Spec(body=Src1 + ((_d > C0) - (_d < (Zero - C0))) * C0, reference=_dmq_ref),
)


# ------------------------------------------------------------ build program
def _build_program(th_val):
    nc = bacc.Bacc(None)
    xin = nc.dram_tensor("xin", [128, S * CW], I16, kind="ExternalInput")
    # rowgroup 0 spikes as fp8 recon-deltas; rowgroup 1 recon as fp16
    d8t = nc.dram_tensor("d8", [128, S * U], FP8, kind="ExternalOutput")
    spk = nc.dram_tensor("spk", [128, S * U], F16, kind="ExternalOutput")
    rwt = nc.dram_tensor("rw", [128, 2 * U], F32, kind="ExternalOutput")
    rlt = nc.dram_tensor("rl", [128, 2 * U], F32, kind="ExternalOutput")

    with tile.TileContext(nc) as tc:
        with (
            tc.tile_pool(name="xpin", bufs=1) as pinpool,
            tc.tile_pool(name="xring", bufs=RBUFS) as ringpool,
            tc.tile_pool(name="kp", bufs=KBUFS) as kpool,
            tc.tile_pool(name="sp", bufs=SBUFS) as spool,
            tc.tile_pool(name="dp", bufs=DBUFS) as dpool,
            tc.tile_pool(name="cp", bufs=1) as cpool,
        ):
            K0 = cpool.tile([128, 2 * U], F32)
            nc.gpsimd.memset(K0[:], 0.0)

            pin = []
            for p in range(NPIN):
                xp = pinpool.tile([128, PL * CW], I16, tag=f"pin{p}", name=f"xp{p}")
                if p == 0 and SPLIT_FIRST:
                    sf = SPLIT_FIRST
                    nc.sync.dma_start(xp[:, 0 : sf * CW], xin[:, 0 : sf * CW])
                    nc.sync.dma_start(
                        xp[:, sf * CW : PL * CW], xin[:, sf * CW : PL * CW]
                    )
                else:
                    nc.sync.dma_start(
                        xp[:], xin[:, p * PL * CW : (p + 1) * PL * CW]
                    )
                pin.append(xp)

            def in0_ap(xt, row, g, off):
                # rowgroup g's U lanes of stream row `row` (contiguous)
                base = row * CW + g * (U + 1) + off
                return xt[:, base : base + U]

            # two interleaved chains (one per rowgroup): each op's true
            # dependency is 2 instructions back, so Tile's completion
            # semaphore (engine + drain + sem-prop, ~95ns) hides under the
            # other chain's engine time instead of serializing every step.
            kprev = [K0[:, 0:U], K0[:, U : 2 * U]]
            kprev_tile = None  # previous K piece (for Pool boundary diff)
            for pc in range(NPIECE):
                i0 = pc * PL
                if i0 < W:
                    xt, off = pin[pc], 0
                elif i0 < S:
                    xt = ringpool.tile([128, PL * CW], I16, tag="xr", name=f"xr{pc}")
                    nc.sync.dma_start(xt[:], xin[:, i0 * CW : (i0 + PL) * CW])
                    off = 0
                else:
                    xt, off = pin[pc - S // PL], 1

                KP = kpool.tile([128, PL * 2 * U], F32, tag="k", name=f"k{pc}")
                for il in range(PL):
                    for g in (0, 1):
                        o0 = il * 2 * U + g * U
                        nc.vector._custom_dve(
                            DM_STEP,
                            out=KP[:, o0 : o0 + U],
                            in0=in0_ap(xt, il, g, off),
                            in1=kprev[g],
                            s0=float(th_val),
                            s1=float(QF),
                        )
                        kprev[g] = KP[:, o0 : o0 + U]

                if i0 + PL == W:
                    # recon entering emit span (step W-1)
                    nc.sync.dma_start(rwt[:], KP[:, (PL - 1) * 2 * U : PL * 2 * U])
                if i0 >= W and ABL_EXTRACT:
                    tl0 = i0 - W
                    ep = (i0 - W) // PL       # emit piece index
                    gsl = ep % OGRP           # slot within output group
                    kv = KP[:].rearrange("p (s l) -> p s l", s=PL)
                    if gsl == 0:
                        D8 = dpool.tile([128, OGRP * PL, U], FP8, tag="d", name=f"d{pc}")
                        SP = spool.tile([128, OGRP * PL, U], F16, tag="s", name=f"s{pc}")
                    r0 = gsl * PL
                    pv = kprev_tile[:].rearrange("p (s l) -> p s l", s=PL)
                    last = False  # tail-split regressed in TimelineSim; disabled
                    # last piece: extract in halves so the first half (and its
                    # DMA) overlaps the final DVE steps; tail = a 2-row chain
                    subs = ((0, PL // 2), (PL // 2, PL)) if last else ((0, PL),)
                    # last piece: diff on the then-idle DVE for a shorter tail
                    deng = nc.vector if pc == NPIECE - 1 else nc.gpsimd
                    for a, b in subs:
                        # rowgroup 0: diff -> fp8
                        if a == 0:
                            nc.gpsimd.tensor_tensor(
                                D8[:, r0 : r0 + 1, :],
                                kv[:, 0:1, 0:U],
                                pv[:, PL - 1 : PL, 0:U],
                                AluOpType.subtract,
                            )
                            lo = 1
                        else:
                            lo = a
                        deng.tensor_tensor(
                            D8[:, r0 + lo : r0 + b, :],
                            kv[:, lo:b, 0:U],
                            kv[:, lo - 1 : b - 1, 0:U],
                            AluOpType.subtract,
                        )
                        # rowgroup 1: recon as fp16 via ACT
                        nc.scalar.activation(
                            SP[:, r0 + a : r0 + b, :],
                            kv[:, a:b, U : 2 * U],
                            mybir.ActivationFunctionType.Copy,
                        )
                        if last:
                            nc.scalar.dma_start(
                                d8t[:, (tl0 + a) * U : (tl0 + b) * U],
                                D8[:, r0 + a : r0 + b, :],
                            )
                            nc.scalar.dma_start(
                                spk[:, (tl0 + a) * U : (tl0 + b) * U],
                                SP[:, r0 + a : r0 + b, :],
                            )
                    if not last and gsl == OGRP - 1:
                        n = (gsl + 1) * PL
                        g0 = tl0 - gsl * PL
                        nc.scalar.dma_start(
                            d8t[:, g0 * U : (g0 + n) * U], D8[:, 0:n, :]
                        )
                        nc.scalar.dma_start(
                            spk[:, g0 * U : (g0 + n) * U], SP[:, 0:n, :]
                        )
                if pc == NPIECE - 1:
                    nc.sync.dma_start(rlt[:], KP[:, (PL - 1) * 2 * U : PL * 2 * U])
                kprev_tile = KP
    nc.finalize()
    return nc


_NC_CACHE = {}


def _get_program(th_val):
    key = float(th_val)
    if key not in _NC_CACHE:
        _NC_CACHE[key] = _build_program(key)
    return _NC_CACHE[key]


# ------------------------------------------------------------ host helpers
def quantize(xs):
    """xs (R, T) f32 -> (int16 codes, dequantized f32 x-hat)."""
    k = np.clip(np.rint(xs * np.float32(2.0 ** QLOG)), -32767, 32767).astype(
        np.int16
    )
    return k, k.astype(np.float32) * QF


def build_xin(k_core):
    """k_core: (256, T) int16 -> xin (128, S*CW) int16.

    xin[p, i*CW + g*(U+1) + j] = kpad[g*128+p, j*S + i], kpad = k_core
    left-padded with W zeros (tail pad never consumed).
    """
    kpad = np.zeros((RPC, W + T + S), dtype=np.int16)
    kpad[:, W : W + T] = k_core
    st_r, st_e = kpad.strides
    A = np.lib.stride_tricks.as_strided(
        kpad, shape=(RPC, U + 1, S), strides=(st_r, S * st_e, st_e)
    )  # A[r, j, i] = kpad[r, j*S + i]
    out = np.empty((128, S, 2, U + 1), dtype=np.int16)
    At = A.transpose(0, 2, 1)  # (r, i, j)
    out[:, :, 0, :] = At[:128]
    out[:, :, 1, :] = At[128:]
    return np.ascontiguousarray(out.reshape(128, S * CW))


def decode_outputs(results, xq, th):
    """results: per-core dicts with 'd8' (fp8), 'spk' (fp16), 'rw','rl' (f32).
    xq: (R, T) f32 dequantized input. Returns the exact x-hat-scan spikes
    (R, T) f32."""
    th = np.float32(th)
    half = th / np.float32(2)
    out = np.empty((R, T), dtype=np.float32)
    rw = np.empty((R, U), dtype=np.float32)
    rl = np.empty((R, U), dtype=np.float32)
    for core in range(N_CORES):
        r = results[core]
        rw2 = np.asarray(r["rw"]).reshape(128, 2, U)
        # rowgroup 0: fp8 recon-deltas
        d8 = np.asarray(r["d8"]).reshape(128, S, U).astype(np.float32)
        s0 = (d8 > half).astype(np.float32) - (d8 < -half).astype(np.float32)
        # rowgroup 1: fp16 recon -> diff
        k16 = np.asarray(r["spk"]).reshape(128, S, U).astype(np.float32)
        d1 = np.empty_like(k16)
        d1[:, 0] = k16[:, 0] - rw2[:, 1]
        d1[:, 1:] = k16[:, 1:] - k16[:, :-1]
        s1 = (d1 > half).astype(np.float32) - (d1 < -half).astype(np.float32)
        blk = out[core * RPC : (core + 1) * RPC].reshape(2, 128, U, S)
        blk[0] = s0.transpose(0, 2, 1)
        blk[1] = s1.transpose(0, 2, 1)
        rw[core * RPC : (core + 1) * RPC] = rw2.transpose(1, 0, 2).reshape(RPC, U)
        rl[core * RPC : (core + 1) * RPC] = (
            np.asarray(r["rl"]).reshape(128, 2, U).transpose(1, 0, 2).reshape(RPC, U)
        )

    # ---- chain-verified fixup (see module docstring): sequential over
    # chunks (vectorized over rows), so cascaded breaks cost one pass.
    rlc = rl[:, 0].copy()  # corrected end state of the previous chunk
    outv = out.reshape(R, U, S)
    for j in range(1, U):
        bad = np.abs(rw[:, j] - rlc) > half
        if bad.any():
            rows = np.nonzero(bad)[0]
            xseg = xq[:, j * S : (j + 1) * S][rows]
            rcur = rlc[rows].copy()
            seg = np.empty((len(rows), S), dtype=np.float32)
            for i in range(S):
                dd = xseg[:, i] - rcur
                net = (dd > th).astype(np.float32) - (dd < -th).astype(np.float32)
                rcur = rcur + net * th
                seg[:, i] = net
            outv[rows, j] = seg
            rlc = rl[:, j].copy()
            rlc[rows] = rcur
        else:
            rlc = rl[:, j]
    return out


# ------------------------------------------------------------------- kernel
def kernel(x, threshold):
    x = np.ascontiguousarray(np.asarray(x, dtype=np.float32))
    th = np.float32(
        min(max(np.float32(threshold), np.float32(0.01)), np.float32(0.5))
    )
    assert x.shape == (B, C, T)

    xs = x.reshape(R, T)
    k, xq = quantize(xs)

    in_maps = []
    for core in range(N_CORES):
        xin = build_xin(k[core * RPC : (core + 1) * RPC])
        in_maps.append({"xin": xin})

    nc = _get_program(th)
    res = run_bass_kernel_spmd(nc, in_maps, list(range(N_CORES)))

    out = decode_outputs(res.results, xq, th)
    return out.reshape(B, C, T)


if __name__ == "__main__":
    rng = np.random.default_rng(0)
    xv = rng.normal(0, 1, (B, C, T)).astype(np.float32)
    o = kernel(x=xv, threshold=np.float32(0.1))
    print("kernel ran; out", o.shape, o.dtype, np.unique(o))
